# revision 12
# baseline (speedup 1.0000x reference)
"""Trainium2 Bass kernel for nn_DepthwiseSeparableFusedConv2d.

Self-contained: takes FULL inputs (x [32,256,56,56] + weights), returns FULL
output [32,256,56,56].  Data-parallel over batch across 8 NeuronCores; the
QuantMeasure / RangeBN global statistics are synchronized with 3 small
AllGather collectives (plus one warm-up).

Core ideas vs a naive port:
 - every quantize step (affine + clip + round) is ONE instruction: the
   fp32->u8 output converter rounds-to-nearest-even and saturates to [0,255],
   exactly matching round(clip(.)) of the reference.
 - depthwise conv runs on the PE at fp16 speed with EXACT integer arithmetic:
   x is fed as integer codes k (exact in fp16), weights as integer codes
   W' = Wint + round(mn_w/s_w); the fractional remainder rides a 10th
   accumulation tap f_frac * Box where Box = 3x3 box-sum of k (computed
   separably on DVE/Pool).  Borders are handled with zero-padding plus
   per-strip constant corrections.
 - pointwise conv in fp16 (integer k2 codes x fp16 scaled weights).
 - BN means come from raw h1/h3 channel sums (accumulated for free in the
   PSUM-evict activations), folded into the stats AllGather.
"""

import math
import numpy as np

# ---------------------------------------------------------------- constants
P = 128
G = 2                 # channel groups (256 = 2*128)
B_FULL = 32
BL = 4                # batches per core
NCORES = 8
HH = 56
IMG = HH * HH         # 3136
PADW = 58
NBLK = 7              # row blocks of 8 rows
BLKW = 8 * HH         # 448 output pixels per block
QMAX = 255.0
N_TOT = B_FULL * IMG  # 100352
NCHUNKS = 16
EPS = 1e-5
_N_CHUNK_EL = B_FULL * IMG // NCHUNKS
SCALE_FIX = float((0.5 * 0.35) * (1 + (math.pi * math.log(4)) ** 0.5)
                  / ((2 * math.log(_N_CHUNK_EL)) ** 0.5))

_PROGRAM_CACHE = {}


def _host_quant_codes(w):
    """Return (codes, s, mn): w_quant = s*codes + mn, codes integer 0..255."""
    w = np.asarray(w, np.float32)
    mn = np.float32(w.min())
    mx = np.float32(w.max())
    s = np.float32(max((mx - mn) / np.float32(QMAX), 1e-8))
    t = np.clip((w - mn) / s, np.float32(0.0), np.float32(QMAX))
    return np.round(t).astype(np.float32), s, mn


def _host_quant(w):
    c, s, mn = _host_quant_codes(w)
    return (c * s + mn).astype(np.float32)


def build_program(limit=7):
    import concourse.bacc as bacc
    import concourse.mybir as mybir
    import concourse.tile as tile

    f32 = mybir.dt.float32
    f16 = mybir.dt.float16
    u8 = mybir.dt.uint8
    AL = mybir.AluOpType
    AF = mybir.ActivationFunctionType
    AX = mybir.AxisListType

    nc = bacc.Bacc('TRN2', target_bir_lowering=False, debug=False,
                   num_devices=NCORES)

    # ------------------------------------------------ external tensors
    x_in = nc.dram_tensor('x', [BL, 256, HH, HH], f32, kind='ExternalInput')
    ident_in = nc.dram_tensor('ident', [P, P], f32, kind='ExternalInput')
    # integer-shifted depthwise weight codes W' = Wint + round(mn_w/s_w)
    wp_in = nc.dram_tensor('wp', [G, P, 9], f32, kind='ExternalInput')
    # packed per-channel consts: [16, G, P] (see _host_consts for order)
    gp_in = nc.dram_tensor('gp', [16, G, P], f32, kind='ExternalInput')
    # pwT[kg, cin(128), (coutg, cout)] : lhsT layout, already transposed
    pwT_in = nc.dram_tensor('pwT', [G, P, 256], f32, kind='ExternalInput')
    fscal_in = nc.dram_tensor('fscal', [1, 2], f32, kind='ExternalInput')
    out_d = nc.dram_tensor('out', [BL, 256, HH, HH], f32, kind='ExternalOutput')

    rg = [list(range(NCORES))]

    with tile.TileContext(nc) as tc:
        with (
            tc.tile_pool(name='perm', bufs=1) as perm,
            tc.tile_pool(name='img', bufs=10) as img,
            tc.tile_pool(name='dram', bufs=1, space='DRAM') as dpool,
            tc.tile_pool(name='tp', bufs=1, space='PSUM') as tpp,
            tc.tile_pool(name='scr', bufs=2) as scrp,
        ):
            # ---------------- constants
            ident = perm.tile([P, P], f32, name='identsb')
            nc.sync.dma_start(ident[:], ident_in[:])
            wp = perm.tile([P, G, 9], f32, name='wpsb')
            nc.sync.dma_start(wp[:], wp_in.rearrange('g c t -> c g t'))
            gpc = perm.tile([P, 16, G], f32, name='gpcsb')
            nc.sync.dma_start(gpc[:], gp_in.rearrange('k g c -> c k g'))
            # order in gp: 0 wsum,1 qdb,2 qbn1w,3 bn1b,4 qbn2w,5 bn2b,6 pwsum,
            # 7 wtop,8 wbot,9 wleft,10 wright,11 w00,12 w02,13 w20,14 w22,
            # 15 csum4
            def gp(i):
                return gpc[:, i]
            pwT = perm.tile([P, G, 256], f32, name='pwTsb')
            nc.sync.dma_start(pwT[:], pwT_in[:].rearrange('g c m -> c g m'))
            fscal = perm.tile([1, 2], f32, name='fscal')  # [f_frac, s_w]
            nc.sync.dma_start(fscal[:], fscal_in[:])

            # fp16 diag weight matrices (value-independent, integer codes)
            diagWp = perm.tile([P, G * 9 * P], f16, name='diagWp')
            for g in range(G):
                for t in range(9):
                    i = g * 9 + t
                    nc.vector.tensor_scalar(
                        diagWp[:, i * P:(i + 1) * P], ident[:],
                        wp[:, g, t:t + 1], None, op0=AL.mult)
            # f_frac * identity (fp16) for the Box tap
            ffrac_bc = perm.tile([P, 1], f32, name='ffrac_bc')
            nc.gpsimd.partition_broadcast(ffrac_bc[:], fscal[:, 0:1])
            factI16 = perm.tile([P, P], f16, name='factI16')
            nc.vector.tensor_scalar(factI16[:], ident[:], ffrac_bc[:, 0:1],
                                    None, op0=AL.mult)

            # ---------------- stat tiles
            xstat = [perm.tile([P, 2 * BL], f32, name=f'xstat{g}')
                     for g in range(G)]
            stat1 = [perm.tile([P, 9], f32, name=f'stat1_{g}')
                     for g in range(G)]
            stat3 = [perm.tile([P, 9], f32, name=f'stat3_{g}')
                     for g in range(G)]
            h1part = [perm.tile([P, 2 * BL], f32, name=f'h1part{g}')
                      for g in range(G)]
            h3part = [perm.tile([P, 2 * BL], f32, name=f'h3part{g}')
                      for g in range(G)]
            Ag = [perm.tile([P, NCORES, 9], f32, name=f'Ag{g}')
                  for g in range(G)]

            def sc1(nm):
                return perm.tile([1, 1], f32, name=nm)

            def bc1(src, nm):
                t = perm.tile([P, 1], f32, name=nm)
                nc.gpsimd.partition_broadcast(t[:], src[:])
                return t

            # scalar math: sum-of-mins/maxes -> quant params (mn, s, 1/s, ...)
            def quant_params(mnsum, mxsum, tag):
                mn = sc1(f'mn_{tag}')
                mx = sc1(f'mx_{tag}')
                nc.vector.tensor_scalar(mn[:], mnsum[:], 1.0 / B_FULL, None,
                                        op0=AL.mult)
                nc.vector.tensor_scalar(mx[:], mxsum[:], 1.0 / B_FULL, None,
                                        op0=AL.mult)
                d = sc1(f'd_{tag}')
                nc.vector.tensor_sub(d[:], mx[:], mn[:])
                s = sc1(f's_{tag}')
                nc.vector.tensor_scalar(s[:], d[:], 1.0 / QMAX, 1e-8,
                                        op0=AL.mult, op1=AL.max)
                inv_s = sc1(f'invs_{tag}')
                nc.vector.reciprocal(inv_s[:], s[:])
                negmn = sc1(f'negmn_{tag}')
                nc.vector.tensor_scalar(negmn[:], mn[:], -1.0, None,
                                        op0=AL.mult)
                bias = sc1(f'bias_{tag}')
                nc.vector.tensor_mul(bias[:], negmn[:], inv_s[:])
                return {'mn': mn, 'mx': mx, 's': s, 'inv_s': inv_s,
                        'negmn': negmn, 'bias': bias}

            # per-sample params from per-(c, core, b) min/max views
            def sample_params(mnviews, mxviews, tag):
                tmn = tpp.tile([B_FULL, G * P], f32, name=f'tmn_{tag}',
                               tag='tp')
                tmx = tpp.tile([B_FULL, G * P], f32, name=f'tmx_{tag}',
                               tag='tp')
                for g in range(G):
                    # transpose needs a contiguous stationary AP
                    cmn = perm.tile([P, B_FULL], f32, name=f'cmn_{tag}{g}')
                    cmx = perm.tile([P, B_FULL], f32, name=f'cmx_{tag}{g}')
                    vmn = cmn.rearrange('p (core b) -> p core b', b=BL)
                    vmx = cmx.rearrange('p (core b) -> p core b', b=BL)
                    nc.vector.tensor_scalar(vmn[:], mnviews[g], 1.0, None,
                                            op0=AL.mult)
                    nc.vector.tensor_scalar(vmx[:], mxviews[g], 1.0, None,
                                            op0=AL.mult)
                    nc.tensor.transpose(tmn[:, g * P:(g + 1) * P],
                                        cmn[:], ident[:])
                    nc.tensor.transpose(tmx[:, g * P:(g + 1) * P],
                                        cmx[:], ident[:])
                pm = perm.tile([B_FULL, 2], f32, name=f'pm_{tag}')
                nc.vector.tensor_reduce(pm[:, 0:1], tmn[:], axis=AX.X,
                                        op=AL.min)
                nc.vector.tensor_reduce(pm[:, 1:2], tmx[:], axis=AX.X,
                                        op=AL.max)
                ta = tpp.tile([1, B_FULL], f32, name=f'ta_{tag}', tag='tp')
                tb = tpp.tile([1, B_FULL], f32, name=f'tb_{tag}', tag='tp')
                nc.tensor.transpose(ta[:], pm[:, 0:1],
                                    ident[0:B_FULL, 0:B_FULL])
                nc.tensor.transpose(tb[:], pm[:, 1:2],
                                    ident[0:B_FULL, 0:B_FULL])
                mnsum = sc1(f'mnsum_{tag}')
                mxsum = sc1(f'mxsum_{tag}')
                nc.vector.tensor_reduce(mnsum[:], ta[:], axis=AX.X, op=AL.add)
                nc.vector.tensor_reduce(mxsum[:], tb[:], axis=AX.X, op=AL.add)
                return quant_params(mnsum, mxsum, tag)

            # =================================================================
            # Phase A: load x + per-(c,b) stats
            # =================================================================
            # folded min/max: elementwise fold of halves, then half reduce
            def stat_minmax(src, dst_min, dst_max, scrpool):
                sc = scrpool.tile([P, IMG // 2], f32, name='scf', tag='scr')
                nc.vector.tensor_tensor(sc[:], src[:, 0:IMG // 2],
                                        src[:, IMG // 2:IMG], op=AL.min)
                nc.vector.tensor_reduce(dst_min, sc[:], axis=AX.X, op=AL.min)
                sc2 = scrpool.tile([P, IMG // 2], f32, name='scf2', tag='scr')
                nc.vector.tensor_tensor(sc2[:], src[:, 0:IMG // 2],
                                        src[:, IMG // 2:IMG], op=AL.max)
                nc.vector.tensor_reduce(dst_max, sc2[:], axis=AX.X, op=AL.max)

            xt = {}
            for b in range(BL):
                for g in range(G):
                    t = img.tile([P, IMG], f32, name=f'x{g}_{b}', tag='img')
                    xt[(g, b)] = t
                    nc.sync.dma_start(
                        t[:], x_in[b, g * P:(g + 1) * P].rearrange(
                            'c h w -> c (h w)'))
                    stat_minmax(t, xstat[g][:, b:b + 1],
                                xstat[g][:, BL + b:BL + b + 1], scrp)

            # ---------------- AG1: per-sample min/max (8 floats per core)
            tmin = tpp.tile([BL, G * P], f32, name='tmin1', tag='tp')
            tmax = tpp.tile([BL, G * P], f32, name='tmax1', tag='tp')
            for g in range(G):
                nc.tensor.transpose(tmin[:, g * P:(g + 1) * P],
                                    xstat[g][:, 0:BL], ident[:])
                nc.tensor.transpose(tmax[:, g * P:(g + 1) * P],
                                    xstat[g][:, BL:2 * BL], ident[:])
            ab1 = perm.tile([BL, 2], f32, name='ab1')
            nc.vector.tensor_reduce(ab1[:, 0:1], tmin[:], axis=AX.X, op=AL.min)
            nc.vector.tensor_reduce(ab1[:, 1:2], tmax[:], axis=AX.X, op=AL.max)
            ag1_in = dpool.tile([BL * 2], f32, name='ag1_in')
            ag1_out = dpool.tile([NCORES * BL * 2], f32, name='ag1_out')
            nc.sync.dma_start(ag1_in.rearrange('(b s) -> b s', s=2), ab1[:])
            nc.gpsimd.collective_compute(
                'AllGather', AL.bypass, replica_groups=rg,
                ins=[ag1_in[:].opt()], outs=[ag1_out[:].opt()])
            agb1 = perm.tile([1, NCORES * BL * 2], f32, name='agb1')
            nc.sync.dma_start(agb1[:], ag1_out[None, :])
            v1 = agb1.rearrange('p (cb s) -> p s cb', s=2)
            mnsum_x = sc1('mnsum_x')
            mxsum_x = sc1('mxsum_x')
            nc.vector.tensor_reduce(mnsum_x[:], v1[:, 0, :], axis=AX.X,
                                    op=AL.add)
            nc.vector.tensor_reduce(mxsum_x[:], v1[:, 1, :], axis=AX.X,
                                    op=AL.add)
            qx = quant_params(mnsum_x, mxsum_x, 'x')
            invsx_bc = bc1(qx['inv_s'], 'invsx_bc')
            biasx_bc = bc1(qx['bias'], 'biasx_bc')
            mnx_bc = bc1(qx['mn'], 'mnx_bc')
            negmnx_bc = bc1(qx['negmn'], 'negmnx_bc')
            # S = s_x * s_w  (depthwise evict scale)
            Ssc = sc1('Ssc')
            swn = perm.tile([1, 1], f32, name='swn')
            nc.vector.tensor_scalar(swn[:], fscal[:, 1:2], 1.0, None,
                                    op0=AL.mult)
            nc.vector.tensor_mul(Ssc[:], qx['s'][:], swn[:])
            Sdw_bc = bc1(Ssc, 'Sdw_bc')

            # strip-correction per-channel constants (computed once)
            cstrip = perm.tile([P, 8, G], f32, name='cstrip')
            # 0..3: -mn*w{top,bot,left,right}; 4..7: +mn*w{00,02,20,22}
            for i, k in enumerate((7, 8, 9, 10)):
                nc.vector.tensor_scalar(cstrip[:, i], gp(k),
                                        negmnx_bc[:, 0:1], None, op0=AL.mult)
            for i, k in enumerate((11, 12, 13, 14)):
                nc.vector.tensor_scalar(cstrip[:, 4 + i], gp(k),
                                        mnx_bc[:, 0:1], None, op0=AL.mult)
            # const1 = qdb + mn*wsum ; sum adjust = mn*csum4
            const1 = perm.tile([P, G], f32, name='const1')
            nc.vector.scalar_tensor_tensor(
                const1[:], gp(0), mnx_bc[:, 0:1], gp(1),
                op0=AL.mult, op1=AL.add)
            adj1 = perm.tile([P, G], f32, name='adj1')
            nc.vector.tensor_scalar(adj1[:], gp(15), mnx_bc[:, 0:1], None,
                                    op0=AL.mult)

            # =================================================================
            # Phase B+C: quantize x -> k (u8->f16), Box, depthwise PE, evict
            # =================================================================
            h1 = {}
            with (
                tc.tile_pool(name='bpool', bufs=3) as bpool,
                tc.tile_pool(name='kxp', bufs=2) as kxp,
                tc.tile_pool(name='cva', bufs=1, space='PSUM') as cvap,
                tc.tile_pool(name='cvb', bufs=1, space='PSUM') as cvbp,
            ):
                deferred = []
                for b in range(BL):
                    for g in range(G):
                        ku = kxp.tile([P, IMG], u8, name=f'ku{g}_{b}',
                                      tag='ku')
                        nc.scalar.activation(ku[:], xt[(g, b)][:], AF.Relu,
                                             bias=biasx_bc[:, 0:1],
                                             scale=invsx_bc[:, 0:1])
                        kh = bpool.tile([P, PADW, PADW], f16,
                                        name=f'kh{g}_{b}', tag='kh')
                        # zero borders (rows 0,57; cols 0,57)
                        nc.gpsimd.memset(kh[:, 0, :], 0.0)
                        nc.gpsimd.memset(kh[:, 57, :], 0.0)
                        nc.gpsimd.memset(kh[:, 1:57, 0], 0.0)
                        nc.gpsimd.memset(kh[:, 1:57, 57], 0.0)
                        nc.scalar.activation(
                            kh[:, 1:57, 1:57],
                            ku.rearrange('p (h w) -> p h w', h=HH),
                            AF.Identity, bias=0.0, scale=1.0)
                        # V = vertical 3-sum of kh; the horizontal 3-sum of
                        # the f_frac*Box term rides the PE as 3 extra taps
                        V = bpool.tile([P, HH, PADW], f16, name=f'V{g}_{b}',
                                       tag='bx')
                        nc.vector.tensor_tensor(V[:], kh[:, 0:56, :],
                                                kh[:, 1:57, :], op=AL.add)
                        nc.vector.tensor_tensor(V[:], V[:], kh[:, 2:58, :],
                                                op=AL.add)
                        box = bpool.tile([P, HH, HH], f16, name=f'bx{g}_{b}',
                                         tag='bx')
                        nc.vector.tensor_tensor(box[:], V[:, :, 0:56],
                                                V[:, :, 1:57], op=AL.add)
                        nc.vector.tensor_tensor(box[:], box[:],
                                                V[:, :, 2:58], op=AL.add)
                        boxf = box.rearrange('p h w -> p (h w)')
                        # depthwise: 7 row-blocks x (9 diag + box tap);
                        # PSUM blocks packed 4+3 into two multi-bank tiles
                        h1t = img.tile([P, IMG], f32, name=f'h1_{g}_{b}',
                                       tag='img')
                        h1[(g, b)] = h1t
                        psa = cvap.tile([P, 4, 512], f32, name=f'cva{g}{b}',
                                        tag='cva')
                        psb = cvbp.tile([P, 3, 512], f32, name=f'cvb{g}{b}',
                                        tag='cvb')
                        for rb in range(NBLK):
                            ps = (psa[:, rb, 0:BLKW] if rb < 4
                                  else psb[:, rb - 4, 0:BLKW])
                            for t in range(9):
                                di, dj = t // 3, t % 3
                                lhs = diagWp[:, (g * 9 + t) * P:
                                             (g * 9 + t + 1) * P]
                                rhs = kh[:, 8 * rb + di:8 * rb + di + 8,
                                         dj:dj + 56]
                                nc.tensor.matmul(ps, lhs, rhs,
                                                 start=(t == 0), stop=False)
                            nc.tensor.matmul(
                                ps, factI16[:],
                                boxf[:, rb * BLKW:(rb + 1) * BLKW],
                                start=False, stop=True)
                        h1v3 = h1t.rearrange('p (r w) -> p r w', w=BLKW)
                        nc.scalar.activation(
                            h1v3[:, 0:4, :], psa[:, :, 0:BLKW],
                            AF.Identity, bias=const1[:, g:g + 1],
                            scale=Sdw_bc[:, 0:1],
                            accum_out=h1part[g][:, 2 * b:2 * b + 1])
                        nc.scalar.activation(
                            h1v3[:, 4:7, :], psb[:, :, 0:BLKW],
                            AF.Identity, bias=const1[:, g:g + 1],
                            scale=Sdw_bc[:, 0:1],
                            accum_out=h1part[g][:, 2 * b + 1:2 * b + 2])

                        def post_c(g=g, b=b, h1t=h1t):
                            # border strip corrections
                            h1v = h1t.rearrange('p (h w) -> p h w', h=HH)
                            nc.vector.tensor_scalar(
                                h1t[:, 0:56], h1t[:, 0:56],
                                cstrip[:, 0, g:g + 1], None, op0=AL.add)
                            nc.vector.tensor_scalar(
                                h1t[:, 3080:3136], h1t[:, 3080:3136],
                                cstrip[:, 1, g:g + 1], None, op0=AL.add)
                            nc.vector.tensor_scalar(
                                h1v[:, :, 0], h1v[:, :, 0],
                                cstrip[:, 2, g:g + 1], None, op0=AL.add)
                            nc.vector.tensor_scalar(
                                h1v[:, :, 55], h1v[:, :, 55],
                                cstrip[:, 3, g:g + 1], None, op0=AL.add)
                            for ci, off in enumerate((0, 55, 3080, 3135)):
                                nc.vector.tensor_scalar(
                                    h1t[:, off:off + 1],
                                    h1t[:, off:off + 1],
                                    cstrip[:, 4 + ci, g:g + 1], None,
                                    op0=AL.add)
                            stat_minmax(h1t, stat1[g][:, b:b + 1],
                                        stat1[g][:, BL + b:BL + b + 1],
                                        scrp)
                        deferred.append(post_c)
                        if len(deferred) > 1:
                            deferred.pop(0)()
                while deferred:
                    deferred.pop(0)()

                # channel sums (+ analytic border-correction adjustment)
                for g in range(G):
                    nc.vector.tensor_reduce(stat1[g][:, 8:9], h1part[g][:],
                                            axis=AX.X, op=AL.add)
                    nc.vector.tensor_tensor(stat1[g][:, 8:9],
                                            stat1[g][:, 8:9],
                                            adj1[:, g:g + 1], op=AL.add)

            # =================================================================
            # AG2: per-(c,b) h1 min/max + per-c sums
            # =================================================================
            ag2_in = dpool.tile([G * P * 9], f32, name='ag2_in')
            ag2_out = dpool.tile([NCORES * G * P * 9], f32, name='ag2_out')
            v2i = ag2_in.rearrange('(g c f) -> g c f', g=G, c=P)
            for g in range(G):
                nc.sync.dma_start(v2i[g], stat1[g][:])
            nc.gpsimd.collective_compute(
                'AllGather', AL.bypass, replica_groups=rg,
                ins=[ag2_in[:].opt()], outs=[ag2_out[:].opt()])
            v2o = ag2_out.rearrange('(core g c f) -> g c core f',
                                    core=NCORES, g=G, c=P)
            for g in range(G):
                nc.sync.dma_start(Ag[g][:], v2o[g])

            # ---- q1 params (per-sample over all 256 channels)
            q1 = sample_params([Ag[g][:, :, 0:BL] for g in range(G)],
                               [Ag[g][:, :, BL:2 * BL] for g in range(G)],
                               'h1')
            invs1_bc = bc1(q1['inv_s'], 'invs1_bc')
            bias1_bc = bc1(q1['bias'], 'bias1_bc')
            s1_bc = bc1(q1['s'], 's1_bc')
            mn1_bc = bc1(q1['mn'], 'mn1_bc')

            # ---- RangeBN scale from chunk (batch-pair) stats
            def rangebn_scale(AgT, invs_bc, bias_bc, s_bc, mn_bc, tag):
                scpk = perm.tile([P, G], f32, name=f'scpk_{tag}')
                for g in range(G):
                    cmin = perm.tile([P, NCHUNKS], f32, name=f'cmin_{tag}{g}')
                    cmax = perm.tile([P, NCHUNKS], f32, name=f'cmax_{tag}{g}')
                    cminv = cmin.rearrange('p (core pr) -> p core pr', pr=2)
                    cmaxv = cmax.rearrange('p (core pr) -> p core pr', pr=2)
                    nc.vector.tensor_tensor(cminv[:], AgT[g][:, :, 0:BL:2],
                                            AgT[g][:, :, 1:BL:2], op=AL.min)
                    nc.vector.tensor_tensor(cmaxv[:],
                                            AgT[g][:, :, BL:2 * BL:2],
                                            AgT[g][:, :, BL + 1:2 * BL:2],
                                            op=AL.max)
                    # quantize chunk stats (monotone): k then value form
                    kq = perm.tile([P, 2 * NCHUNKS], u8, name=f'kq_{tag}{g}')
                    nc.scalar.activation(kq[:, 0:NCHUNKS], cmin[:], AF.Relu,
                                         bias=bias_bc[:, 0:1],
                                         scale=invs_bc[:, 0:1])
                    nc.scalar.activation(kq[:, NCHUNKS:], cmax[:], AF.Relu,
                                         bias=bias_bc[:, 0:1],
                                         scale=invs_bc[:, 0:1])
                    vq = perm.tile([P, 2 * NCHUNKS], f32, name=f'vq_{tag}{g}')
                    nc.vector.tensor_scalar(vq[:], kq[:], s_bc[:, 0:1],
                                            mn_bc[:, 0:1], op0=AL.mult,
                                            op1=AL.add)
                    mm = perm.tile([P, 2], f32, name=f'mm_{tag}{g}')
                    nc.vector.tensor_reduce(mm[:, 0:1], vq[:, 0:NCHUNKS],
                                            axis=AX.X, op=AL.add)
                    nc.vector.tensor_reduce(mm[:, 1:2], vq[:, NCHUNKS:],
                                            axis=AX.X, op=AL.add)
                    d = perm.tile([P, 1], f32, name=f'dmm_{tag}{g}')
                    nc.vector.tensor_sub(d[:], mm[:, 1:2], mm[:, 0:1])
                    # d = (mean_max-mean_min)*scale_fix + eps  (note /16)
                    nc.vector.tensor_scalar(d[:], d[:],
                                            SCALE_FIX / NCHUNKS, EPS,
                                            op0=AL.mult, op1=AL.add)
                    nc.vector.reciprocal(scpk[:, g:g + 1], d[:])
                # per-tensor quantize of the 256 scales
                tq = tpp.tile([1, G * P], f32, name=f'tq_{tag}', tag='tp')
                for g in range(G):
                    nc.tensor.transpose(tq[:, g * P:(g + 1) * P],
                                        scpk[:, g:g + 1], ident[:])
                smn = sc1(f'smn_{tag}')
                smx = sc1(f'smx_{tag}')
                nc.vector.tensor_reduce(smn[:], tq[:], axis=AX.X, op=AL.min)
                nc.vector.tensor_reduce(smx[:], tq[:], axis=AX.X, op=AL.max)
                dd = sc1(f'sd_{tag}')
                nc.vector.tensor_sub(dd[:], smx[:], smn[:])
                ss = sc1(f'ss_{tag}')
                nc.vector.tensor_scalar(ss[:], dd[:], 1.0 / QMAX, 1e-8,
                                        op0=AL.mult, op1=AL.max)
                invss = sc1(f'invss_{tag}')
                nc.vector.reciprocal(invss[:], ss[:])
                negsmn = sc1(f'negsmn_{tag}')
                nc.vector.tensor_scalar(negsmn[:], smn[:], -1.0, None,
                                        op0=AL.mult)
                bss = sc1(f'bss_{tag}')
                nc.vector.tensor_mul(bss[:], negsmn[:], invss[:])
                invss_bc = bc1(invss, f'invss_bc_{tag}')
                bss_bc = bc1(bss, f'bss_bc_{tag}')
                ss_bc = bc1(ss, f'ss_bc_{tag}')
                smn_bc = bc1(smn, f'smn_bc_{tag}')
                kqs = perm.tile([P, G], u8, name=f'kqs_{tag}')
                nc.scalar.activation(kqs[:], scpk[:], AF.Relu,
                                     bias=bss_bc[:, 0:1],
                                     scale=invss_bc[:, 0:1])
                nc.vector.tensor_scalar(scpk[:], kqs[:], ss_bc[:, 0:1],
                                        smn_bc[:, 0:1], op0=AL.mult,
                                        op1=AL.add)
                return scpk

            qscale1 = rangebn_scale(Ag, invs1_bc, bias1_bc, s1_bc, mn1_bc,
                                    'bn1')
            A1 = perm.tile([P, G], f32, name='A1')
            nc.vector.tensor_mul(A1[:], qscale1[:], gp(2))
            cA1 = perm.tile([P, G], f32, name='cA1')
            nc.vector.tensor_scalar(cA1[:], A1[:], s1_bc[:, 0:1], None,
                                    op0=AL.mult)
            # mean1 = (sum over cores of per-core h1 sums) / N_TOT
            mean1 = perm.tile([P, G], f32, name='mean1')
            for g in range(G):
                nc.vector.tensor_reduce(mean1[:, g:g + 1], Ag[g][:, :, 8],
                                        axis=AX.X, op=AL.add)
            nc.vector.tensor_scalar(mean1[:], mean1[:], 1.0 / N_TOT, None,
                                    op0=AL.mult)
            cB1 = perm.tile([P, G], f32, name='cB1')
            nc.vector.tensor_scalar(cB1[:], mean1[:], -1.0, mn1_bc[:, 0:1],
                                    op0=AL.mult, op1=AL.add)
            nc.vector.tensor_mul(cB1[:], cB1[:], A1[:])
            nc.vector.tensor_add(cB1[:], cB1[:], gp(3))

            # ---- analytic qm(h2) bounds: transform Ag min/max in place
            for g in range(G):
                kb = perm.tile([P, 2 * BL * NCORES], u8, name=f'kb_{g}')
                kbv = kb.rearrange('p (core f) -> p core f', f=2 * BL)
                nc.scalar.activation(kbv[:], Ag[g][:, :, 0:2 * BL], AF.Relu,
                                     bias=bias1_bc[:, 0:1],
                                     scale=invs1_bc[:, 0:1])
                nc.scalar.activation(Ag[g][:, :, 0:2 * BL], kbv[:], AF.Relu,
                                     bias=cB1[:, g:g + 1],
                                     scale=cA1[:, g:g + 1])
            q2 = sample_params([Ag[g][:, :, 0:BL] for g in range(G)],
                               [Ag[g][:, :, BL:2 * BL] for g in range(G)],
                               'h2')
            invs2_bc = bc1(q2['inv_s'], 'invs2_bc')
            mn2_bc = bc1(q2['mn'], 'mn2_bc')
            s2_bc = bc1(q2['s'], 's2_bc')

            # fused D/E coefficients and pointwise weight prep
            a2t = perm.tile([P, G], f32, name='a2t')
            nc.vector.tensor_scalar(a2t[:], cA1[:], invs2_bc[:, 0:1], None,
                                    op0=AL.mult)
            b2t = perm.tile([P, G], f32, name='b2t')
            nc.vector.tensor_scalar(b2t[:], cB1[:], mn2_bc[:, 0:1],
                                    invs2_bc[:, 0:1], op0=AL.subtract,
                                    op1=AL.mult)
            pwTs = perm.tile([P, G, 256], f16, name='pwTs')
            nc.scalar.activation(pwTs[:], pwT[:], AF.Identity, bias=0.0,
                                 scale=s2_bc[:, 0:1])
            const3 = perm.tile([P, G], f32, name='const3')
            nc.vector.tensor_scalar(const3[:], gp(6), mn2_bc[:, 0:1], None,
                                    op0=AL.mult)

            # =================================================================
            # Phase D/E/F: h1 -> k1 -> k2 -> pointwise -> h3 (in SBUF)
            # =================================================================
            h3 = {}
            with (
                tc.tile_pool(name='k1p', bufs=3) as k1p,
                tc.tile_pool(name='k2up', bufs=2) as k2up,
                tc.tile_pool(name='k2fp', bufs=4) as k2fp,
                tc.tile_pool(name='pwa', bufs=1, space='PSUM') as pwap,
                tc.tile_pool(name='pwb', bufs=1, space='PSUM') as pwbp,
            ):
                deferred3 = []
                for b in range(BL):
                    k2f = {}
                    for g in range(G):
                        k1t = k1p.tile([P, IMG], u8, name=f'k1_{g}_{b}',
                                       tag='k1')
                        if g == 0:
                            nc.scalar.activation(k1t[:], h1[(g, b)][:],
                                                 AF.Relu,
                                                 bias=bias1_bc[:, 0:1],
                                                 scale=invs1_bc[:, 0:1])
                        else:
                            nc.vector.tensor_scalar(k1t[:], h1[(g, b)][:],
                                                    invs1_bc[:, 0:1],
                                                    bias1_bc[:, 0:1],
                                                    op0=AL.mult, op1=AL.add)
                        k2u = k2up.tile([P, IMG], u8, name=f'k2u_{g}_{b}',
                                        tag='k2u')
                        nc.scalar.activation(k2u[:], k1t[:], AF.Relu,
                                             bias=b2t[:, g:g + 1],
                                             scale=a2t[:, g:g + 1])
                        k2ft = k2fp.tile([P, IMG], f16, name=f'k2f_{g}_{b}',
                                         tag='k2f')
                        k2f[g] = k2ft
                        nc.vector.tensor_scalar(k2ft[:], k2u[:], 1.0, None,
                                                op0=AL.mult)
                    for cg in range(G):
                        h3t = img.tile([P, IMG], f32, name=f'h3_{cg}_{b}',
                                       tag='img')
                        h3[(cg, b)] = h3t
                        psa = pwap.tile([P, 4, 512], f32, name=f'pwa{cg}{b}',
                                        tag='pwa')
                        psb = pwbp.tile([P, 3, 512], f32, name=f'pwb{cg}{b}',
                                        tag='pwb')
                        for rb in range(NBLK):
                            ps = (psa[:, rb, 0:BLKW] if rb < 4
                                  else psb[:, rb - 4, 0:BLKW])
                            for kg in range(G):
                                lhs = pwTs[:, kg, cg * P:(cg + 1) * P]
                                nc.tensor.matmul(
                                    ps, lhs,
                                    k2f[kg][:, rb * BLKW:(rb + 1) * BLKW],
                                    start=(kg == 0), stop=(kg == 1))
                        h3v3 = h3t.rearrange('p (r w) -> p r w', w=BLKW)
                        nc.scalar.activation(
                            h3v3[:, 0:4, :], psa[:, :, 0:BLKW],
                            AF.Identity, bias=const3[:, cg:cg + 1],
                            scale=1.0,
                            accum_out=h3part[cg][:, 2 * b:2 * b + 1])
                        nc.scalar.activation(
                            h3v3[:, 4:7, :], psb[:, :, 0:BLKW],
                            AF.Identity, bias=const3[:, cg:cg + 1],
                            scale=1.0,
                            accum_out=h3part[cg][:, 2 * b + 1:2 * b + 2])

                        def post_f(cg=cg, b=b, h3t=h3t):
                            stat_minmax(h3t, stat3[cg][:, b:b + 1],
                                        stat3[cg][:, BL + b:BL + b + 1],
                                        scrp)
                        deferred3.append(post_f)
                        if len(deferred3) > 1:
                            deferred3.pop(0)()
                while deferred3:
                    deferred3.pop(0)()
                for g in range(G):
                    nc.vector.tensor_reduce(stat3[g][:, 8:9], h3part[g][:],
                                            axis=AX.X, op=AL.add)

            # =================================================================
            # AG5 + BN2 math
            # =================================================================
            ag5_in = dpool.tile([G * P * 9], f32, name='ag5_in')
            ag5_out = dpool.tile([NCORES * G * P * 9], f32, name='ag5_out')
            v5i = ag5_in.rearrange('(g c f) -> g c f', g=G, c=P)
            for g in range(G):
                nc.sync.dma_start(v5i[g], stat3[g][:])
            nc.gpsimd.collective_compute(
                'AllGather', AL.bypass, replica_groups=rg,
                ins=[ag5_in[:].opt()], outs=[ag5_out[:].opt()])
            v5o = ag5_out.rearrange('(core g c f) -> g c core f',
                                    core=NCORES, g=G, c=P)
            for g in range(G):
                nc.sync.dma_start(Ag[g][:], v5o[g])

            q3 = sample_params([Ag[g][:, :, 0:BL] for g in range(G)],
                               [Ag[g][:, :, BL:2 * BL] for g in range(G)],
                               'h3')
            invs3_bc = bc1(q3['inv_s'], 'invs3_bc')
            bias3_bc = bc1(q3['bias'], 'bias3_bc')
            s3_bc = bc1(q3['s'], 's3_bc')
            mn3_bc = bc1(q3['mn'], 'mn3_bc')
            qscale3 = rangebn_scale(Ag, invs3_bc, bias3_bc, s3_bc, mn3_bc,
                                    'bn2')
            A3 = perm.tile([P, G], f32, name='A3')
            nc.vector.tensor_mul(A3[:], qscale3[:], gp(4))
            cA3 = perm.tile([P, G], f32, name='cA3')
            nc.vector.tensor_scalar(cA3[:], A3[:], s3_bc[:, 0:1], None,
                                    op0=AL.mult)
            mean3 = perm.tile([P, G], f32, name='mean3')
            for g in range(G):
                nc.vector.tensor_reduce(mean3[:, g:g + 1], Ag[g][:, :, 8],
                                        axis=AX.X, op=AL.add)
            nc.vector.tensor_scalar(mean3[:], mean3[:], 1.0 / N_TOT, None,
                                    op0=AL.mult)
            cB3 = perm.tile([P, G], f32, name='cB3')
            nc.vector.tensor_scalar(cB3[:], mean3[:], -1.0, mn3_bc[:, 0:1],
                                    op0=AL.mult, op1=AL.add)
            nc.vector.tensor_mul(cB3[:], cB3[:], A3[:])
            nc.vector.tensor_add(cB3[:], cB3[:], gp(5))

            # =================================================================
            # Phase G/H: h3 -> k3 -> out
            # =================================================================
            with tc.tile_pool(name='k3p', bufs=3) as k3p:
                for b in range(BL):
                    for cg in range(G):
                        k3t = k3p.tile([P, IMG], u8, name=f'k3_{cg}_{b}',
                                       tag='k3')
                        nc.vector.tensor_scalar(k3t[:], h3[(cg, b)][:],
                                                invs3_bc[:, 0:1],
                                                bias3_bc[:, 0:1],
                                                op0=AL.mult, op1=AL.add)
                        ot = img.tile([P, IMG], f32, name=f'out_{cg}_{b}',
                                      tag='img')
                        nc.scalar.activation(ot[:], k3t[:], AF.Relu,
                                             bias=cB3[:, cg:cg + 1],
                                             scale=cA3[:, cg:cg + 1])
                        nc.sync.dma_start(
                            out_d[b, cg * P:(cg + 1) * P].rearrange(
                                'c h w -> c (h w)'), ot[:])

    nc.compile()
    return nc


def _host_consts(dw_w, dw_b, bn1_w, bn1_b, pw_w, bn2_w, bn2_b):
    wcodes, s_w, mn_w = _host_quant_codes(np.asarray(dw_w).reshape(256, 9))
    qdw = (wcodes * s_w + mn_w).astype(np.float32)
    f = float(mn_w) / float(s_w)
    f_int = float(np.round(f))
    f_frac = np.float32(f - f_int)
    wp = (wcodes + np.float32(f_int)).astype(np.float32)

    qdb = _host_quant(dw_b)
    qpw = _host_quant(np.asarray(pw_w).reshape(256, 256))
    qbn1w = _host_quant(bn1_w)
    qbn2w = _host_quant(bn2_w)
    wsum = qdw.sum(axis=1, dtype=np.float32)
    wtop = qdw[:, 0:3].sum(axis=1, dtype=np.float32)
    wbot = qdw[:, 6:9].sum(axis=1, dtype=np.float32)
    wleft = qdw[:, (0, 3, 6)].sum(axis=1, dtype=np.float32)
    wright = qdw[:, (2, 5, 8)].sum(axis=1, dtype=np.float32)
    w00, w02, w20, w22 = qdw[:, 0], qdw[:, 2], qdw[:, 6], qdw[:, 8]
    csum4 = (4.0 * (-56.0 * (wtop + wbot + wleft + wright)
                    + (w00 + w02 + w20 + w22))).astype(np.float32)
    pwsum = qpw.sum(axis=1, dtype=np.float32)
    pwT = np.ascontiguousarray(qpw.T.reshape(G, P, 256)).astype(np.float32)

    def gpr(a):
        return np.asarray(a, np.float32).reshape(G, P)

    gpk = np.stack([gpr(wsum), gpr(qdb), gpr(qbn1w), gpr(bn1_b),
                    gpr(qbn2w), gpr(bn2_b), gpr(pwsum), gpr(wtop),
                    gpr(wbot), gpr(wleft), gpr(wright), gpr(w00),
                    gpr(w02), gpr(w20), gpr(w22), gpr(csum4)], axis=0)
    return {
        'ident': np.eye(P, dtype=np.float32),
        'wp': np.ascontiguousarray(wp.reshape(G, P, 9)),
        'gp': np.ascontiguousarray(gpk),
        'pwT': pwT,
        'fscal': np.array([[f_frac, s_w]], dtype=np.float32),
    }


def make_in_maps(x, dw_w, dw_b, bn1_w, bn1_b, pw_w, bn2_w, bn2_b):
    x = np.asarray(x, np.float32)
    consts = _host_consts(dw_w, dw_b, bn1_w, bn1_b, pw_w, bn2_w, bn2_b)
    in_maps = []
    for c in range(NCORES):
        m = dict(consts)
        m['x'] = np.ascontiguousarray(x[c * BL:(c + 1) * BL])
        in_maps.append(m)
    return in_maps


def get_program(limit=7):
    if limit not in _PROGRAM_CACHE:
        _PROGRAM_CACHE[limit] = build_program(limit)
    return _PROGRAM_CACHE[limit]


def kernel(**inputs):
    from concourse.bass_utils import run_bass_kernel_spmd
    nc = get_program()
    in_maps = make_in_maps(**inputs)
    res = run_bass_kernel_spmd(nc, in_maps, core_ids=list(range(NCORES)))
    out = np.concatenate([res.results[i]['out'] for i in range(NCORES)],
                         axis=0)
    return out.astype(np.float32)


# revision 16
# speedup vs baseline: 1.0413x; 1.0413x over previous
"""Trainium2 Bass kernel for nn_DepthwiseSeparableFusedConv2d.

Self-contained: takes FULL inputs (x [32,256,56,56] + weights), returns FULL
output [32,256,56,56].  Data-parallel over batch across 8 NeuronCores; the
QuantMeasure / RangeBN global statistics are synchronized with 3 small
AllGather collectives (plus one warm-up).

Core ideas vs a naive port:
 - every quantize step (affine + clip + round) is ONE instruction: the
   fp32->u8 output converter rounds-to-nearest-even and saturates to [0,255],
   exactly matching round(clip(.)) of the reference.
 - depthwise conv runs on the PE at fp16 speed with EXACT integer arithmetic:
   x is fed as integer codes k (exact in fp16), weights as integer codes
   W' = Wint + round(mn_w/s_w); the fractional remainder rides a 10th
   accumulation tap f_frac * Box where Box = 3x3 box-sum of k (computed
   separably on DVE/Pool).  Borders are handled with zero-padding plus
   per-strip constant corrections.
 - pointwise conv in fp16 (integer k2 codes x fp16 scaled weights).
 - BN means come from raw h1/h3 channel sums (accumulated for free in the
   PSUM-evict activations), folded into the stats AllGather.
"""

import math
import numpy as np

# ---------------------------------------------------------------- constants
P = 128
G = 2                 # channel groups (256 = 2*128)
B_FULL = 32
BL = 4                # batches per core
NCORES = 8
HH = 56
IMG = HH * HH         # 3136
PADW = 58
NBLK = 7              # row blocks of 8 rows
BLKW = 8 * HH         # 448 output pixels per block
QMAX = 255.0
N_TOT = B_FULL * IMG  # 100352
NCHUNKS = 16
EPS = 1e-5
_N_CHUNK_EL = B_FULL * IMG // NCHUNKS
SCALE_FIX = float((0.5 * 0.35) * (1 + (math.pi * math.log(4)) ** 0.5)
                  / ((2 * math.log(_N_CHUNK_EL)) ** 0.5))

_PROGRAM_CACHE = {}


def _host_quant_codes(w):
    """Return (codes, s, mn): w_quant = s*codes + mn, codes integer 0..255."""
    w = np.asarray(w, np.float32)
    mn = np.float32(w.min())
    mx = np.float32(w.max())
    s = np.float32(max((mx - mn) / np.float32(QMAX), 1e-8))
    t = np.clip((w - mn) / s, np.float32(0.0), np.float32(QMAX))
    return np.round(t).astype(np.float32), s, mn


def _host_quant(w):
    c, s, mn = _host_quant_codes(w)
    return (c * s + mn).astype(np.float32)


def build_program(limit=7):
    import concourse.bacc as bacc
    import concourse.mybir as mybir
    import concourse.tile as tile

    f32 = mybir.dt.float32
    f16 = mybir.dt.float16
    u8 = mybir.dt.uint8
    AL = mybir.AluOpType
    AF = mybir.ActivationFunctionType
    AX = mybir.AxisListType

    nc = bacc.Bacc('TRN2', target_bir_lowering=False, debug=False,
                   num_devices=NCORES)

    # ------------------------------------------------ external tensors
    x_in = nc.dram_tensor('x', [BL, 256, HH, HH], f32, kind='ExternalInput')
    ident_in = nc.dram_tensor('ident', [P, P], f32, kind='ExternalInput')
    # integer-shifted depthwise weight codes W' = Wint + round(mn_w/s_w)
    wp_in = nc.dram_tensor('wp', [G, P, 9], f32, kind='ExternalInput')
    # packed per-channel consts: [16, G, P] (see _host_consts for order)
    gp_in = nc.dram_tensor('gp', [16, G, P], f32, kind='ExternalInput')
    # pwT[kg, cin(128), (coutg, cout)] : lhsT layout, already transposed
    pwT_in = nc.dram_tensor('pwT', [G, P, 256], f32, kind='ExternalInput')
    fscal_in = nc.dram_tensor('fscal', [1, 2], f32, kind='ExternalInput')
    out_d = nc.dram_tensor('out', [BL, 256, HH, HH], f32, kind='ExternalOutput')

    rg = [list(range(NCORES))]

    with tile.TileContext(nc) as tc:
        with (
            tc.tile_pool(name='perm', bufs=1) as perm,
            tc.tile_pool(name='img', bufs=10) as img,
            tc.tile_pool(name='dram', bufs=1, space='DRAM') as dpool,
            tc.tile_pool(name='tp', bufs=1, space='PSUM') as tpp,
            tc.tile_pool(name='scr', bufs=2) as scrp,
        ):
            # ---------------- warm-up collective (absorbs first-cc cost)
            ag0_in = dpool.tile([8], f32, name='ag0_in')
            ag0_out = dpool.tile([8 * NCORES], f32, name='ag0_out')
            nc.gpsimd.collective_compute(
                'AllGather', AL.bypass, replica_groups=rg,
                ins=[ag0_in[:].opt()], outs=[ag0_out[:].opt()])

            # ---------------- constants
            ident = perm.tile([P, P], f32, name='identsb')
            nc.sync.dma_start(ident[:], ident_in[:])
            wp = perm.tile([P, G, 9], f32, name='wpsb')
            nc.sync.dma_start(wp[:], wp_in.rearrange('g c t -> c g t'))
            gpc = perm.tile([P, 16, G], f32, name='gpcsb')
            nc.sync.dma_start(gpc[:], gp_in.rearrange('k g c -> c k g'))
            # order in gp: 0 wsum,1 qdb,2 qbn1w,3 bn1b,4 qbn2w,5 bn2b,6 pwsum,
            # 7 wtop,8 wbot,9 wleft,10 wright,11 w00,12 w02,13 w20,14 w22,
            # 15 csum4
            def gp(i):
                return gpc[:, i]
            pwT = perm.tile([P, G, 256], f32, name='pwTsb')
            nc.sync.dma_start(pwT[:], pwT_in[:].rearrange('g c m -> c g m'))
            fscal = perm.tile([1, 2], f32, name='fscal')  # [f_frac, s_w]
            nc.sync.dma_start(fscal[:], fscal_in[:])

            # fp16 diag weight matrices (value-independent, integer codes)
            diagWp = perm.tile([P, G * 9 * P], f16, name='diagWp')
            for g in range(G):
                for t in range(9):
                    i = g * 9 + t
                    nc.vector.tensor_scalar(
                        diagWp[:, i * P:(i + 1) * P], ident[:],
                        wp[:, g, t:t + 1], None, op0=AL.mult)
            # f_frac * identity (fp16) for the Box tap
            ffrac_bc = perm.tile([P, 1], f32, name='ffrac_bc')
            nc.gpsimd.partition_broadcast(ffrac_bc[:], fscal[:, 0:1])
            factI16 = perm.tile([P, P], f16, name='factI16')
            nc.vector.tensor_scalar(factI16[:], ident[:], ffrac_bc[:, 0:1],
                                    None, op0=AL.mult)

            # ---------------- stat tiles
            xstat = [perm.tile([P, 2 * BL], f32, name=f'xstat{g}')
                     for g in range(G)]
            stat1 = [perm.tile([P, 9], f32, name=f'stat1_{g}')
                     for g in range(G)]
            stat3 = [perm.tile([P, 9], f32, name=f'stat3_{g}')
                     for g in range(G)]
            h1part = [perm.tile([P, 4 * BL], f32, name=f'h1part{g}')
                      for g in range(G)]
            h3part = [perm.tile([P, 4 * BL], f32, name=f'h3part{g}')
                      for g in range(G)]
            Ag = [perm.tile([P, NCORES, 9], f32, name=f'Ag{g}')
                  for g in range(G)]

            def sc1(nm):
                return perm.tile([1, 1], f32, name=nm)

            def bc1(src, nm):
                t = perm.tile([P, 1], f32, name=nm)
                nc.gpsimd.partition_broadcast(t[:], src[:])
                return t

            # scalar math: sum-of-mins/maxes -> quant params (mn, s, 1/s, ...)
            def quant_params(mnsum, mxsum, tag):
                mn = sc1(f'mn_{tag}')
                mx = sc1(f'mx_{tag}')
                nc.vector.tensor_scalar(mn[:], mnsum[:], 1.0 / B_FULL, None,
                                        op0=AL.mult)
                nc.vector.tensor_scalar(mx[:], mxsum[:], 1.0 / B_FULL, None,
                                        op0=AL.mult)
                d = sc1(f'd_{tag}')
                nc.vector.tensor_sub(d[:], mx[:], mn[:])
                s = sc1(f's_{tag}')
                nc.vector.tensor_scalar(s[:], d[:], 1.0 / QMAX, 1e-8,
                                        op0=AL.mult, op1=AL.max)
                inv_s = sc1(f'invs_{tag}')
                nc.vector.reciprocal(inv_s[:], s[:])
                negmn = sc1(f'negmn_{tag}')
                nc.vector.tensor_scalar(negmn[:], mn[:], -1.0, None,
                                        op0=AL.mult)
                bias = sc1(f'bias_{tag}')
                nc.vector.tensor_mul(bias[:], negmn[:], inv_s[:])
                return {'mn': mn, 'mx': mx, 's': s, 'inv_s': inv_s,
                        'negmn': negmn, 'bias': bias}

            # per-sample params from per-(c, core, b) min/max views
            def sample_params(mnviews, mxviews, tag):
                tmn = tpp.tile([B_FULL, G * P], f32, name=f'tmn_{tag}',
                               tag='tp')
                tmx = tpp.tile([B_FULL, G * P], f32, name=f'tmx_{tag}',
                               tag='tp')
                for g in range(G):
                    cmn = perm.tile([P, B_FULL], f32, name=f'cmn_{tag}{g}')
                    cmx = perm.tile([P, B_FULL], f32, name=f'cmx_{tag}{g}')
                    vmn = cmn.rearrange('p (core b) -> p core b', b=BL)
                    vmx = cmx.rearrange('p (core b) -> p core b', b=BL)
                    nc.vector.tensor_scalar(vmn[:], mnviews[g], 1.0, None,
                                            op0=AL.mult)
                    nc.vector.tensor_scalar(vmx[:], mxviews[g], 1.0, None,
                                            op0=AL.mult)
                    nc.tensor.transpose(tmn[:, g * P:(g + 1) * P],
                                        cmn[:], ident[:])
                    nc.tensor.transpose(tmx[:, g * P:(g + 1) * P],
                                        cmx[:], ident[:])
                pm = perm.tile([B_FULL, 2], f32, name=f'pm_{tag}')
                nc.vector.tensor_reduce(pm[:, 0:1], tmn[:], axis=AX.X,
                                        op=AL.min)
                nc.vector.tensor_reduce(pm[:, 1:2], tmx[:], axis=AX.X,
                                        op=AL.max)
                ta = tpp.tile([1, B_FULL], f32, name=f'ta_{tag}', tag='tp')
                tb = tpp.tile([1, B_FULL], f32, name=f'tb_{tag}', tag='tp')
                nc.tensor.transpose(ta[:], pm[:, 0:1],
                                    ident[0:B_FULL, 0:B_FULL])
                nc.tensor.transpose(tb[:], pm[:, 1:2],
                                    ident[0:B_FULL, 0:B_FULL])
                mnsum = sc1(f'mnsum_{tag}')
                mxsum = sc1(f'mxsum_{tag}')
                nc.vector.tensor_reduce(mnsum[:], ta[:], axis=AX.X, op=AL.add)
                nc.vector.tensor_reduce(mxsum[:], tb[:], axis=AX.X, op=AL.add)
                return quant_params(mnsum, mxsum, tag)

            # =================================================================
            # Phase A: load x + per-(c,b) stats
            # =================================================================
            # folded min/max: elementwise fold of halves, then half reduce
            def stat_minmax(src, dst_min, dst_max, scrpool):
                sc = scrpool.tile([P, IMG // 2], f32, name='scf', tag='scr')
                nc.vector.tensor_tensor(sc[:], src[:, 0:IMG // 2],
                                        src[:, IMG // 2:IMG], op=AL.min)
                nc.vector.tensor_reduce(dst_min, sc[:], axis=AX.X, op=AL.min)
                sc2 = scrpool.tile([P, IMG // 2], f32, name='scf2', tag='scr')
                nc.vector.tensor_tensor(sc2[:], src[:, 0:IMG // 2],
                                        src[:, IMG // 2:IMG], op=AL.max)
                nc.vector.tensor_reduce(dst_max, sc2[:], axis=AX.X, op=AL.max)

            xt = {}
            for b in range(BL):
                for g in range(G):
                    t = img.tile([P, IMG], f32, name=f'x{g}_{b}', tag='img')
                    xt[(g, b)] = t
                    nc.sync.dma_start(
                        t[:], x_in[b, g * P:(g + 1) * P].rearrange(
                            'c h w -> c (h w)'))
                    stat_minmax(t, xstat[g][:, b:b + 1],
                                xstat[g][:, BL + b:BL + b + 1], scrp)

            # ---------------- AG1: per-sample min/max (8 floats per core)
            tmin = tpp.tile([BL, G * P], f32, name='tmin1', tag='tp')
            tmax = tpp.tile([BL, G * P], f32, name='tmax1', tag='tp')
            for g in range(G):
                nc.tensor.transpose(tmin[:, g * P:(g + 1) * P],
                                    xstat[g][:, 0:BL], ident[:])
                nc.tensor.transpose(tmax[:, g * P:(g + 1) * P],
                                    xstat[g][:, BL:2 * BL], ident[:])
            ab1 = perm.tile([BL, 2], f32, name='ab1')
            nc.vector.tensor_reduce(ab1[:, 0:1], tmin[:], axis=AX.X, op=AL.min)
            nc.vector.tensor_reduce(ab1[:, 1:2], tmax[:], axis=AX.X, op=AL.max)
            ag1_in = dpool.tile([BL * 2], f32, name='ag1_in')
            ag1_out = dpool.tile([NCORES * BL * 2], f32, name='ag1_out')
            nc.sync.dma_start(ag1_in.rearrange('(b s) -> b s', s=2), ab1[:])
            nc.gpsimd.collective_compute(
                'AllGather', AL.bypass, replica_groups=rg,
                ins=[ag1_in[:].opt()], outs=[ag1_out[:].opt()])
            agb1 = perm.tile([1, NCORES * BL * 2], f32, name='agb1')
            nc.sync.dma_start(agb1[:], ag1_out[None, :])
            v1 = agb1.rearrange('p (cb s) -> p s cb', s=2)
            mnsum_x = sc1('mnsum_x')
            mxsum_x = sc1('mxsum_x')
            nc.vector.tensor_reduce(mnsum_x[:], v1[:, 0, :], axis=AX.X,
                                    op=AL.add)
            nc.vector.tensor_reduce(mxsum_x[:], v1[:, 1, :], axis=AX.X,
                                    op=AL.add)
            qx = quant_params(mnsum_x, mxsum_x, 'x')
            invsx_bc = bc1(qx['inv_s'], 'invsx_bc')
            biasx_bc = bc1(qx['bias'], 'biasx_bc')
            mnx_bc = bc1(qx['mn'], 'mnx_bc')
            negmnx_bc = bc1(qx['negmn'], 'negmnx_bc')
            # S = s_x * s_w  (depthwise evict scale)
            Ssc = sc1('Ssc')
            swn = perm.tile([1, 1], f32, name='swn')
            nc.vector.tensor_scalar(swn[:], fscal[:, 1:2], 1.0, None,
                                    op0=AL.mult)
            nc.vector.tensor_mul(Ssc[:], qx['s'][:], swn[:])
            Sdw_bc = bc1(Ssc, 'Sdw_bc')

            # strip-correction per-channel constants (computed once)
            cstrip = perm.tile([P, 8, G], f32, name='cstrip')
            # 0..3: -mn*w{top,bot,left,right}; 4..7: +mn*w{00,02,20,22}
            for i, k in enumerate((7, 8, 9, 10)):
                nc.vector.tensor_scalar(cstrip[:, i], gp(k),
                                        negmnx_bc[:, 0:1], None, op0=AL.mult)
            for i, k in enumerate((11, 12, 13, 14)):
                nc.vector.tensor_scalar(cstrip[:, 4 + i], gp(k),
                                        mnx_bc[:, 0:1], None, op0=AL.mult)
            # const1 = qdb + mn*wsum ; sum adjust = mn*csum4
            const1 = perm.tile([P, G], f32, name='const1')
            nc.vector.scalar_tensor_tensor(
                const1[:], gp(0), mnx_bc[:, 0:1], gp(1),
                op0=AL.mult, op1=AL.add)
            adj1 = perm.tile([P, G], f32, name='adj1')
            nc.vector.tensor_scalar(adj1[:], gp(15), mnx_bc[:, 0:1], None,
                                    op0=AL.mult)

            # =================================================================
            # Phase B+C: quantize x -> k (u8->f16), Box, depthwise PE, evict
            # =================================================================
            h1 = {}
            with (
                tc.tile_pool(name='bpool', bufs=3) as bpool,
                tc.tile_pool(name='kxp', bufs=2) as kxp,
                tc.tile_pool(name='cva', bufs=3, space='PSUM') as cvap,
                tc.tile_pool(name='cvb', bufs=1, space='PSUM') as cvbp,
            ):
                deferred = []
                for b in range(BL):
                    for g in range(G):
                        ku = kxp.tile([P, IMG], u8, name=f'ku{g}_{b}',
                                      tag='ku')
                        nc.scalar.activation(ku[:], xt[(g, b)][:], AF.Relu,
                                             bias=biasx_bc[:, 0:1],
                                             scale=invsx_bc[:, 0:1])
                        kh = bpool.tile([P, PADW, PADW], f16,
                                        name=f'kh{g}_{b}', tag='kh')
                        # zero borders (rows 0,57; cols 0,57)
                        nc.gpsimd.memset(kh[:, 0, :], 0.0)
                        nc.gpsimd.memset(kh[:, 57, :], 0.0)
                        nc.gpsimd.memset(kh[:, 1:57, 0], 0.0)
                        nc.gpsimd.memset(kh[:, 1:57, 57], 0.0)
                        nc.scalar.activation(
                            kh[:, 1:57, 1:57],
                            ku.rearrange('p (h w) -> p h w', h=HH),
                            AF.Identity, bias=0.0, scale=1.0)
                        # V = vertical 3-sum of kh; the horizontal 3-sum of
                        # the f_frac*Box term rides the PE as 3 extra taps
                        V = bpool.tile([P, HH, PADW], f16, name=f'V{g}_{b}',
                                       tag='bx')
                        nc.vector.tensor_tensor(V[:], kh[:, 0:56, :],
                                                kh[:, 1:57, :], op=AL.add)
                        nc.vector.tensor_tensor(V[:], V[:], kh[:, 2:58, :],
                                                op=AL.add)
                        box = bpool.tile([P, HH, HH], f16, name=f'bx{g}_{b}',
                                         tag='bx')
                        nc.vector.tensor_tensor(box[:], V[:, :, 0:56],
                                                V[:, :, 1:57], op=AL.add)
                        nc.vector.tensor_tensor(box[:], box[:],
                                                V[:, :, 2:58], op=AL.add)
                        boxf = box.rearrange('p h w -> p (h w)')
                        # depthwise: 7 row-blocks x (9 diag + box tap);
                        # PSUM blocks packed 4+3 into two multi-bank tiles
                        h1t = img.tile([P, IMG], f32, name=f'h1_{g}_{b}',
                                       tag='img')
                        h1[(g, b)] = h1t
                        pst = [cvap.tile([P, 2, 512], f32,
                                         name=f'cva{g}{b}{i}', tag='cva')
                               for i in range(3)]
                        psd = cvbp.tile([P, 512], f32, name=f'cvb{g}{b}',
                                        tag='cvb')
                        for rb in range(NBLK):
                            ps = (pst[rb // 2][:, rb % 2, 0:BLKW] if rb < 6
                                  else psd[:, 0:BLKW])
                            for t in range(9):
                                di, dj = t // 3, t % 3
                                lhs = diagWp[:, (g * 9 + t) * P:
                                             (g * 9 + t + 1) * P]
                                rhs = kh[:, 8 * rb + di:8 * rb + di + 8,
                                         dj:dj + 56]
                                nc.tensor.matmul(ps, lhs, rhs,
                                                 start=(t == 0), stop=False)
                            nc.tensor.matmul(
                                ps, factI16[:],
                                boxf[:, rb * BLKW:(rb + 1) * BLKW],
                                start=False, stop=True)
                        h1v3 = h1t.rearrange('p (r w) -> p r w', w=BLKW)
                        for i in range(3):
                            nc.scalar.activation(
                                h1v3[:, 2 * i:2 * i + 2, :],
                                pst[i][:, :, 0:BLKW],
                                AF.Identity, bias=const1[:, g:g + 1],
                                scale=Sdw_bc[:, 0:1],
                                accum_out=h1part[g][:, 4 * b + i:
                                                    4 * b + i + 1])
                        nc.scalar.activation(
                            h1v3[:, 6:7, :], psd[:, 0:BLKW],
                            AF.Identity, bias=const1[:, g:g + 1],
                            scale=Sdw_bc[:, 0:1],
                            accum_out=h1part[g][:, 4 * b + 3:4 * b + 4])

                        def post_c(g=g, b=b, h1t=h1t):
                            # border strip corrections
                            h1v = h1t.rearrange('p (h w) -> p h w', h=HH)
                            nc.vector.tensor_scalar(
                                h1t[:, 0:56], h1t[:, 0:56],
                                cstrip[:, 0, g:g + 1], None, op0=AL.add)
                            nc.vector.tensor_scalar(
                                h1t[:, 3080:3136], h1t[:, 3080:3136],
                                cstrip[:, 1, g:g + 1], None, op0=AL.add)
                            nc.vector.tensor_scalar(
                                h1v[:, :, 0], h1v[:, :, 0],
                                cstrip[:, 2, g:g + 1], None, op0=AL.add)
                            nc.vector.tensor_scalar(
                                h1v[:, :, 55], h1v[:, :, 55],
                                cstrip[:, 3, g:g + 1], None, op0=AL.add)
                            for ci, off in enumerate((0, 55, 3080, 3135)):
                                nc.vector.tensor_scalar(
                                    h1t[:, off:off + 1],
                                    h1t[:, off:off + 1],
                                    cstrip[:, 4 + ci, g:g + 1], None,
                                    op0=AL.add)
                            stat_minmax(h1t, stat1[g][:, b:b + 1],
                                        stat1[g][:, BL + b:BL + b + 1],
                                        scrp)
                        deferred.append(post_c)
                        if len(deferred) > 1:
                            deferred.pop(0)()
                while deferred:
                    deferred.pop(0)()

                # channel sums (+ analytic border-correction adjustment)
                for g in range(G):
                    nc.vector.tensor_reduce(stat1[g][:, 8:9], h1part[g][:],
                                            axis=AX.X, op=AL.add)
                    nc.vector.tensor_tensor(stat1[g][:, 8:9],
                                            stat1[g][:, 8:9],
                                            adj1[:, g:g + 1], op=AL.add)

            # =================================================================
            # AG2: per-(c,b) h1 min/max + per-c sums
            # =================================================================
            ag2_in = dpool.tile([G * P * 9], f32, name='ag2_in')
            ag2_out = dpool.tile([NCORES * G * P * 9], f32, name='ag2_out')
            v2i = ag2_in.rearrange('(g c f) -> g c f', g=G, c=P)
            for g in range(G):
                nc.sync.dma_start(v2i[g], stat1[g][:])
            nc.gpsimd.collective_compute(
                'AllGather', AL.bypass, replica_groups=rg,
                ins=[ag2_in[:].opt()], outs=[ag2_out[:].opt()])
            v2o = ag2_out.rearrange('(core g c f) -> g c core f',
                                    core=NCORES, g=G, c=P)
            for g in range(G):
                nc.sync.dma_start(Ag[g][:], v2o[g])

            # ---- q1 params (per-sample over all 256 channels)
            q1 = sample_params([Ag[g][:, :, 0:BL] for g in range(G)],
                               [Ag[g][:, :, BL:2 * BL] for g in range(G)],
                               'h1')
            invs1_bc = bc1(q1['inv_s'], 'invs1_bc')
            bias1_bc = bc1(q1['bias'], 'bias1_bc')
            s1_bc = bc1(q1['s'], 's1_bc')
            mn1_bc = bc1(q1['mn'], 'mn1_bc')

            # ---- RangeBN scale from chunk (batch-pair) stats
            def rangebn_scale(AgT, invs_bc, bias_bc, s_bc, mn_bc, tag):
                scpk = perm.tile([P, G], f32, name=f'scpk_{tag}')
                for g in range(G):
                    cmin = perm.tile([P, NCHUNKS], f32, name=f'cmin_{tag}{g}')
                    cmax = perm.tile([P, NCHUNKS], f32, name=f'cmax_{tag}{g}')
                    cminv = cmin.rearrange('p (core pr) -> p core pr', pr=2)
                    cmaxv = cmax.rearrange('p (core pr) -> p core pr', pr=2)
                    nc.vector.tensor_tensor(cminv[:], AgT[g][:, :, 0:BL:2],
                                            AgT[g][:, :, 1:BL:2], op=AL.min)
                    nc.vector.tensor_tensor(cmaxv[:],
                                            AgT[g][:, :, BL:2 * BL:2],
                                            AgT[g][:, :, BL + 1:2 * BL:2],
                                            op=AL.max)
                    # quantize chunk stats (monotone): k then value form
                    kq = perm.tile([P, 2 * NCHUNKS], u8, name=f'kq_{tag}{g}')
                    nc.scalar.activation(kq[:, 0:NCHUNKS], cmin[:], AF.Relu,
                                         bias=bias_bc[:, 0:1],
                                         scale=invs_bc[:, 0:1])
                    nc.scalar.activation(kq[:, NCHUNKS:], cmax[:], AF.Relu,
                                         bias=bias_bc[:, 0:1],
                                         scale=invs_bc[:, 0:1])
                    vq = perm.tile([P, 2 * NCHUNKS], f32, name=f'vq_{tag}{g}')
                    nc.vector.tensor_scalar(vq[:], kq[:], s_bc[:, 0:1],
                                            mn_bc[:, 0:1], op0=AL.mult,
                                            op1=AL.add)
                    mm = perm.tile([P, 2], f32, name=f'mm_{tag}{g}')
                    nc.vector.tensor_reduce(mm[:, 0:1], vq[:, 0:NCHUNKS],
                                            axis=AX.X, op=AL.add)
                    nc.vector.tensor_reduce(mm[:, 1:2], vq[:, NCHUNKS:],
                                            axis=AX.X, op=AL.add)
                    d = perm.tile([P, 1], f32, name=f'dmm_{tag}{g}')
                    nc.vector.tensor_sub(d[:], mm[:, 1:2], mm[:, 0:1])
                    # d = (mean_max-mean_min)*scale_fix + eps  (note /16)
                    nc.vector.tensor_scalar(d[:], d[:],
                                            SCALE_FIX / NCHUNKS, EPS,
                                            op0=AL.mult, op1=AL.add)
                    nc.vector.reciprocal(scpk[:, g:g + 1], d[:])
                # per-tensor quantize of the 256 scales
                tq = tpp.tile([1, G * P], f32, name=f'tq_{tag}', tag='tp')
                for g in range(G):
                    nc.tensor.transpose(tq[:, g * P:(g + 1) * P],
                                        scpk[:, g:g + 1], ident[:])
                smn = sc1(f'smn_{tag}')
                smx = sc1(f'smx_{tag}')
                nc.vector.tensor_reduce(smn[:], tq[:], axis=AX.X, op=AL.min)
                nc.vector.tensor_reduce(smx[:], tq[:], axis=AX.X, op=AL.max)
                dd = sc1(f'sd_{tag}')
                nc.vector.tensor_sub(dd[:], smx[:], smn[:])
                ss = sc1(f'ss_{tag}')
                nc.vector.tensor_scalar(ss[:], dd[:], 1.0 / QMAX, 1e-8,
                                        op0=AL.mult, op1=AL.max)
                invss = sc1(f'invss_{tag}')
                nc.vector.reciprocal(invss[:], ss[:])
                negsmn = sc1(f'negsmn_{tag}')
                nc.vector.tensor_scalar(negsmn[:], smn[:], -1.0, None,
                                        op0=AL.mult)
                bss = sc1(f'bss_{tag}')
                nc.vector.tensor_mul(bss[:], negsmn[:], invss[:])
                invss_bc = bc1(invss, f'invss_bc_{tag}')
                bss_bc = bc1(bss, f'bss_bc_{tag}')
                ss_bc = bc1(ss, f'ss_bc_{tag}')
                smn_bc = bc1(smn, f'smn_bc_{tag}')
                kqs = perm.tile([P, G], u8, name=f'kqs_{tag}')
                nc.scalar.activation(kqs[:], scpk[:], AF.Relu,
                                     bias=bss_bc[:, 0:1],
                                     scale=invss_bc[:, 0:1])
                nc.vector.tensor_scalar(scpk[:], kqs[:], ss_bc[:, 0:1],
                                        smn_bc[:, 0:1], op0=AL.mult,
                                        op1=AL.add)
                return scpk

            qscale1 = rangebn_scale(Ag, invs1_bc, bias1_bc, s1_bc, mn1_bc,
                                    'bn1')
            A1 = perm.tile([P, G], f32, name='A1')
            nc.vector.tensor_mul(A1[:], qscale1[:], gp(2))
            cA1 = perm.tile([P, G], f32, name='cA1')
            nc.vector.tensor_scalar(cA1[:], A1[:], s1_bc[:, 0:1], None,
                                    op0=AL.mult)
            # mean1 = (sum over cores of per-core h1 sums) / N_TOT
            mean1 = perm.tile([P, G], f32, name='mean1')
            for g in range(G):
                nc.vector.tensor_reduce(mean1[:, g:g + 1], Ag[g][:, :, 8],
                                        axis=AX.X, op=AL.add)
            nc.vector.tensor_scalar(mean1[:], mean1[:], 1.0 / N_TOT, None,
                                    op0=AL.mult)
            cB1 = perm.tile([P, G], f32, name='cB1')
            nc.vector.tensor_scalar(cB1[:], mean1[:], -1.0, mn1_bc[:, 0:1],
                                    op0=AL.mult, op1=AL.add)
            nc.vector.tensor_mul(cB1[:], cB1[:], A1[:])
            nc.vector.tensor_add(cB1[:], cB1[:], gp(3))

            # ---- analytic qm(h2) bounds: transform Ag min/max in place
            for g in range(G):
                kb = perm.tile([P, 2 * BL * NCORES], u8, name=f'kb_{g}')
                kbv = kb.rearrange('p (core f) -> p core f', f=2 * BL)
                nc.scalar.activation(kbv[:], Ag[g][:, :, 0:2 * BL], AF.Relu,
                                     bias=bias1_bc[:, 0:1],
                                     scale=invs1_bc[:, 0:1])
                nc.scalar.activation(Ag[g][:, :, 0:2 * BL], kbv[:], AF.Relu,
                                     bias=cB1[:, g:g + 1],
                                     scale=cA1[:, g:g + 1])
            q2 = sample_params([Ag[g][:, :, 0:BL] for g in range(G)],
                               [Ag[g][:, :, BL:2 * BL] for g in range(G)],
                               'h2')
            invs2_bc = bc1(q2['inv_s'], 'invs2_bc')
            mn2_bc = bc1(q2['mn'], 'mn2_bc')
            s2_bc = bc1(q2['s'], 's2_bc')

            # fused D/E coefficients and pointwise weight prep
            a2t = perm.tile([P, G], f32, name='a2t')
            nc.vector.tensor_scalar(a2t[:], cA1[:], invs2_bc[:, 0:1], None,
                                    op0=AL.mult)
            b2t = perm.tile([P, G], f32, name='b2t')
            nc.vector.tensor_scalar(b2t[:], cB1[:], mn2_bc[:, 0:1],
                                    invs2_bc[:, 0:1], op0=AL.subtract,
                                    op1=AL.mult)
            pwTs = perm.tile([P, G, 256], f16, name='pwTs')
            nc.scalar.activation(pwTs[:], pwT[:], AF.Identity, bias=0.0,
                                 scale=s2_bc[:, 0:1])
            const3 = perm.tile([P, G], f32, name='const3')
            nc.vector.tensor_scalar(const3[:], gp(6), mn2_bc[:, 0:1], None,
                                    op0=AL.mult)

            # =================================================================
            # Phase D/E/F: h1 -> k1 -> k2 -> pointwise -> h3 (in SBUF)
            # =================================================================
            h3 = {}
            with (
                tc.tile_pool(name='k1p', bufs=3) as k1p,
                tc.tile_pool(name='k2up', bufs=2) as k2up,
                tc.tile_pool(name='k2fp', bufs=4) as k2fp,
                tc.tile_pool(name='pwa', bufs=3, space='PSUM') as pwap,
                tc.tile_pool(name='pwb', bufs=1, space='PSUM') as pwbp,
            ):
                deferred3 = []
                for b in range(BL):
                    k2f = {}
                    for g in range(G):
                        k1t = k1p.tile([P, IMG], u8, name=f'k1_{g}_{b}',
                                       tag='k1')
                        if g == 0:
                            nc.scalar.activation(k1t[:], h1[(g, b)][:],
                                                 AF.Relu,
                                                 bias=bias1_bc[:, 0:1],
                                                 scale=invs1_bc[:, 0:1])
                        else:
                            nc.vector.tensor_scalar(k1t[:], h1[(g, b)][:],
                                                    invs1_bc[:, 0:1],
                                                    bias1_bc[:, 0:1],
                                                    op0=AL.mult, op1=AL.add)
                        k2u = k2up.tile([P, IMG], u8, name=f'k2u_{g}_{b}',
                                        tag='k2u')
                        nc.scalar.activation(k2u[:], k1t[:], AF.Relu,
                                             bias=b2t[:, g:g + 1],
                                             scale=a2t[:, g:g + 1])
                        k2ft = k2fp.tile([P, IMG], f16, name=f'k2f_{g}_{b}',
                                         tag='k2f')
                        k2f[g] = k2ft
                        nc.vector.tensor_scalar(k2ft[:], k2u[:], 1.0, None,
                                                op0=AL.mult)
                    for cg in range(G):
                        h3t = img.tile([P, IMG], f32, name=f'h3_{cg}_{b}',
                                       tag='img')
                        h3[(cg, b)] = h3t
                        pst = [pwap.tile([P, 2, 512], f32,
                                         name=f'pwa{cg}{b}{i}', tag='pwa')
                               for i in range(3)]
                        psd = pwbp.tile([P, 512], f32, name=f'pwb{cg}{b}',
                                        tag='pwb')
                        for rb in range(NBLK):
                            ps = (pst[rb // 2][:, rb % 2, 0:BLKW] if rb < 6
                                  else psd[:, 0:BLKW])
                            for kg in range(G):
                                lhs = pwTs[:, kg, cg * P:(cg + 1) * P]
                                nc.tensor.matmul(
                                    ps, lhs,
                                    k2f[kg][:, rb * BLKW:(rb + 1) * BLKW],
                                    start=(kg == 0), stop=(kg == 1))
                        h3v3 = h3t.rearrange('p (r w) -> p r w', w=BLKW)
                        for i in range(3):
                            nc.scalar.activation(
                                h3v3[:, 2 * i:2 * i + 2, :],
                                pst[i][:, :, 0:BLKW],
                                AF.Identity, bias=const3[:, cg:cg + 1],
                                scale=1.0,
                                accum_out=h3part[cg][:, 4 * b + i:
                                                     4 * b + i + 1])
                        nc.scalar.activation(
                            h3v3[:, 6:7, :], psd[:, 0:BLKW],
                            AF.Identity, bias=const3[:, cg:cg + 1],
                            scale=1.0,
                            accum_out=h3part[cg][:, 4 * b + 3:4 * b + 4])

                        def post_f(cg=cg, b=b, h3t=h3t):
                            stat_minmax(h3t, stat3[cg][:, b:b + 1],
                                        stat3[cg][:, BL + b:BL + b + 1],
                                        scrp)
                        deferred3.append(post_f)
                        if len(deferred3) > 1:
                            deferred3.pop(0)()
                while deferred3:
                    deferred3.pop(0)()
                for g in range(G):
                    nc.vector.tensor_reduce(stat3[g][:, 8:9], h3part[g][:],
                                            axis=AX.X, op=AL.add)

            # =================================================================
            # AG5 + BN2 math
            # =================================================================
            ag5_in = dpool.tile([G * P * 9], f32, name='ag5_in')
            ag5_out = dpool.tile([NCORES * G * P * 9], f32, name='ag5_out')
            v5i = ag5_in.rearrange('(g c f) -> g c f', g=G, c=P)
            for g in range(G):
                nc.sync.dma_start(v5i[g], stat3[g][:])
            nc.gpsimd.collective_compute(
                'AllGather', AL.bypass, replica_groups=rg,
                ins=[ag5_in[:].opt()], outs=[ag5_out[:].opt()])
            v5o = ag5_out.rearrange('(core g c f) -> g c core f',
                                    core=NCORES, g=G, c=P)
            for g in range(G):
                nc.sync.dma_start(Ag[g][:], v5o[g])

            q3 = sample_params([Ag[g][:, :, 0:BL] for g in range(G)],
                               [Ag[g][:, :, BL:2 * BL] for g in range(G)],
                               'h3')
            invs3_bc = bc1(q3['inv_s'], 'invs3_bc')
            bias3_bc = bc1(q3['bias'], 'bias3_bc')
            s3_bc = bc1(q3['s'], 's3_bc')
            mn3_bc = bc1(q3['mn'], 'mn3_bc')
            qscale3 = rangebn_scale(Ag, invs3_bc, bias3_bc, s3_bc, mn3_bc,
                                    'bn2')
            A3 = perm.tile([P, G], f32, name='A3')
            nc.vector.tensor_mul(A3[:], qscale3[:], gp(4))
            cA3 = perm.tile([P, G], f32, name='cA3')
            nc.vector.tensor_scalar(cA3[:], A3[:], s3_bc[:, 0:1], None,
                                    op0=AL.mult)
            mean3 = perm.tile([P, G], f32, name='mean3')
            for g in range(G):
                nc.vector.tensor_reduce(mean3[:, g:g + 1], Ag[g][:, :, 8],
                                        axis=AX.X, op=AL.add)
            nc.vector.tensor_scalar(mean3[:], mean3[:], 1.0 / N_TOT, None,
                                    op0=AL.mult)
            cB3 = perm.tile([P, G], f32, name='cB3')
            nc.vector.tensor_scalar(cB3[:], mean3[:], -1.0, mn3_bc[:, 0:1],
                                    op0=AL.mult, op1=AL.add)
            nc.vector.tensor_mul(cB3[:], cB3[:], A3[:])
            nc.vector.tensor_add(cB3[:], cB3[:], gp(5))

            # =================================================================
            # Phase G/H: h3 -> k3 -> out
            # =================================================================
            with tc.tile_pool(name='k3p', bufs=3) as k3p:
                for b in range(BL):
                    for cg in range(G):
                        k3t = k3p.tile([P, IMG], u8, name=f'k3_{cg}_{b}',
                                       tag='k3')
                        nc.vector.tensor_scalar(k3t[:], h3[(cg, b)][:],
                                                invs3_bc[:, 0:1],
                                                bias3_bc[:, 0:1],
                                                op0=AL.mult, op1=AL.add)
                        ot = img.tile([P, IMG], f32, name=f'out_{cg}_{b}',
                                      tag='img')
                        nc.scalar.activation(ot[:], k3t[:], AF.Relu,
                                             bias=cB3[:, cg:cg + 1],
                                             scale=cA3[:, cg:cg + 1])
                        nc.sync.dma_start(
                            out_d[b, cg * P:(cg + 1) * P].rearrange(
                                'c h w -> c (h w)'), ot[:])

    nc.compile()
    return nc


def _host_consts(dw_w, dw_b, bn1_w, bn1_b, pw_w, bn2_w, bn2_b):
    wcodes, s_w, mn_w = _host_quant_codes(np.asarray(dw_w).reshape(256, 9))
    qdw = (wcodes * s_w + mn_w).astype(np.float32)
    f = float(mn_w) / float(s_w)
    f_int = float(np.round(f))
    f_frac = np.float32(f - f_int)
    wp = (wcodes + np.float32(f_int)).astype(np.float32)

    qdb = _host_quant(dw_b)
    qpw = _host_quant(np.asarray(pw_w).reshape(256, 256))
    qbn1w = _host_quant(bn1_w)
    qbn2w = _host_quant(bn2_w)
    wsum = qdw.sum(axis=1, dtype=np.float32)
    wtop = qdw[:, 0:3].sum(axis=1, dtype=np.float32)
    wbot = qdw[:, 6:9].sum(axis=1, dtype=np.float32)
    wleft = qdw[:, (0, 3, 6)].sum(axis=1, dtype=np.float32)
    wright = qdw[:, (2, 5, 8)].sum(axis=1, dtype=np.float32)
    w00, w02, w20, w22 = qdw[:, 0], qdw[:, 2], qdw[:, 6], qdw[:, 8]
    csum4 = (4.0 * (-56.0 * (wtop + wbot + wleft + wright)
                    + (w00 + w02 + w20 + w22))).astype(np.float32)
    pwsum = qpw.sum(axis=1, dtype=np.float32)
    pwT = np.ascontiguousarray(qpw.T.reshape(G, P, 256)).astype(np.float32)

    def gpr(a):
        return np.asarray(a, np.float32).reshape(G, P)

    gpk = np.stack([gpr(wsum), gpr(qdb), gpr(qbn1w), gpr(bn1_b),
                    gpr(qbn2w), gpr(bn2_b), gpr(pwsum), gpr(wtop),
                    gpr(wbot), gpr(wleft), gpr(wright), gpr(w00),
                    gpr(w02), gpr(w20), gpr(w22), gpr(csum4)], axis=0)
    return {
        'ident': np.eye(P, dtype=np.float32),
        'wp': np.ascontiguousarray(wp.reshape(G, P, 9)),
        'gp': np.ascontiguousarray(gpk),
        'pwT': pwT,
        'fscal': np.array([[f_frac, s_w]], dtype=np.float32),
    }


def make_in_maps(x, dw_w, dw_b, bn1_w, bn1_b, pw_w, bn2_w, bn2_b):
    x = np.asarray(x, np.float32)
    consts = _host_consts(dw_w, dw_b, bn1_w, bn1_b, pw_w, bn2_w, bn2_b)
    in_maps = []
    for c in range(NCORES):
        m = dict(consts)
        m['x'] = np.ascontiguousarray(x[c * BL:(c + 1) * BL])
        in_maps.append(m)
    return in_maps


def get_program(limit=7):
    if limit not in _PROGRAM_CACHE:
        _PROGRAM_CACHE[limit] = build_program(limit)
    return _PROGRAM_CACHE[limit]


def kernel(**inputs):
    from concourse.bass_utils import run_bass_kernel_spmd
    nc = get_program()
    in_maps = make_in_maps(**inputs)
    res = run_bass_kernel_spmd(nc, in_maps, core_ids=list(range(NCORES)))
    out = np.concatenate([res.results[i]['out'] for i in range(NCORES)],
                         axis=0)
    return out.astype(np.float32)


# revision 17
# speedup vs baseline: 1.0417x; 1.0004x over previous
"""Trainium2 Bass kernel for nn_DepthwiseSeparableFusedConv2d.

Self-contained: takes FULL inputs (x [32,256,56,56] + weights), returns FULL
output [32,256,56,56].  Data-parallel over batch across 8 NeuronCores; the
QuantMeasure / RangeBN global statistics are synchronized with 3 small
AllGather collectives (plus one warm-up).

Core ideas vs a naive port:
 - every quantize step (affine + clip + round) is ONE instruction: the
   fp32->u8 output converter rounds-to-nearest-even and saturates to [0,255],
   exactly matching round(clip(.)) of the reference.
 - depthwise conv runs on the PE at fp16 speed with EXACT integer arithmetic:
   x is fed as integer codes k (exact in fp16), weights as integer codes
   W' = Wint + round(mn_w/s_w); the fractional remainder rides a 10th
   accumulation tap f_frac * Box where Box = 3x3 box-sum of k (computed
   separably on DVE/Pool).  Borders are handled with zero-padding plus
   per-strip constant corrections.
 - pointwise conv in fp16 (integer k2 codes x fp16 scaled weights).
 - BN means come from raw h1/h3 channel sums (accumulated for free in the
   PSUM-evict activations), folded into the stats AllGather.
"""

import math
import numpy as np

# ---------------------------------------------------------------- constants
P = 128
G = 2                 # channel groups (256 = 2*128)
B_FULL = 32
BL = 4                # batches per core
NCORES = 8
HH = 56
IMG = HH * HH         # 3136
PADW = 58
NBLK = 7              # row blocks of 8 rows
BLKW = 8 * HH         # 448 output pixels per block
QMAX = 255.0
N_TOT = B_FULL * IMG  # 100352
NCHUNKS = 16
EPS = 1e-5
_N_CHUNK_EL = B_FULL * IMG // NCHUNKS
SCALE_FIX = float((0.5 * 0.35) * (1 + (math.pi * math.log(4)) ** 0.5)
                  / ((2 * math.log(_N_CHUNK_EL)) ** 0.5))

_PROGRAM_CACHE = {}


def _host_quant_codes(w):
    """Return (codes, s, mn): w_quant = s*codes + mn, codes integer 0..255."""
    w = np.asarray(w, np.float32)
    mn = np.float32(w.min())
    mx = np.float32(w.max())
    s = np.float32(max((mx - mn) / np.float32(QMAX), 1e-8))
    t = np.clip((w - mn) / s, np.float32(0.0), np.float32(QMAX))
    return np.round(t).astype(np.float32), s, mn


def _host_quant(w):
    c, s, mn = _host_quant_codes(w)
    return (c * s + mn).astype(np.float32)


def build_program(limit=7):
    import concourse.bacc as bacc
    import concourse.mybir as mybir
    import concourse.tile as tile

    f32 = mybir.dt.float32
    f16 = mybir.dt.float16
    u8 = mybir.dt.uint8
    AL = mybir.AluOpType
    AF = mybir.ActivationFunctionType
    AX = mybir.AxisListType

    nc = bacc.Bacc('TRN2', target_bir_lowering=False, debug=False,
                   num_devices=NCORES)

    # ------------------------------------------------ external tensors
    x_in = nc.dram_tensor('x', [BL, 256, HH, HH], f32, kind='ExternalInput')
    ident_in = nc.dram_tensor('ident', [P, P], f32, kind='ExternalInput')
    # integer-shifted depthwise weight codes W' = Wint + round(mn_w/s_w)
    wp_in = nc.dram_tensor('wp', [G, P, 9], f32, kind='ExternalInput')
    # packed per-channel consts: [16, G, P] (see _host_consts for order)
    gp_in = nc.dram_tensor('gp', [16, G, P], f32, kind='ExternalInput')
    # pwT[kg, cin(128), (coutg, cout)] : lhsT layout, already transposed
    pwT_in = nc.dram_tensor('pwT', [G, P, 256], f32, kind='ExternalInput')
    fscal_in = nc.dram_tensor('fscal', [1, 2], f32, kind='ExternalInput')
    out_d = nc.dram_tensor('out', [BL, 256, HH, HH], f32, kind='ExternalOutput')

    rg = [list(range(NCORES))]

    with tile.TileContext(nc) as tc:
        with (
            tc.tile_pool(name='perm', bufs=1) as perm,
            tc.tile_pool(name='img', bufs=10) as img,
            tc.tile_pool(name='dram', bufs=1, space='DRAM') as dpool,
            tc.tile_pool(name='tp', bufs=1, space='PSUM') as tpp,
            tc.tile_pool(name='scr', bufs=2) as scrp,
        ):
            # ---------------- warm-up collective (absorbs first-cc cost)
            ag0_in = dpool.tile([8], f32, name='ag0_in')
            ag0_out = dpool.tile([8 * NCORES], f32, name='ag0_out')
            nc.gpsimd.collective_compute(
                'AllGather', AL.bypass, replica_groups=rg,
                ins=[ag0_in[:].opt()], outs=[ag0_out[:].opt()])

            # ---------------- constants
            ident = perm.tile([P, P], f32, name='identsb')
            nc.sync.dma_start(ident[:], ident_in[:])
            wp = perm.tile([P, G, 9], f32, name='wpsb')
            nc.sync.dma_start(wp[:], wp_in.rearrange('g c t -> c g t'))
            gpc = perm.tile([P, 16, G], f32, name='gpcsb')
            nc.sync.dma_start(gpc[:], gp_in.rearrange('k g c -> c k g'))
            # order in gp: 0 wsum,1 qdb,2 qbn1w,3 bn1b,4 qbn2w,5 bn2b,6 pwsum,
            # 7 wtop,8 wbot,9 wleft,10 wright,11 w00,12 w02,13 w20,14 w22,
            # 15 csum4
            def gp(i):
                return gpc[:, i]
            pwT = perm.tile([P, G, 256], f32, name='pwTsb')
            nc.sync.dma_start(pwT[:], pwT_in[:].rearrange('g c m -> c g m'))
            fscal = perm.tile([1, 2], f32, name='fscal')  # [f_frac, s_w]
            nc.sync.dma_start(fscal[:], fscal_in[:])

            # fp16 diag weight matrices (value-independent, integer codes)
            diagWp = perm.tile([P, G * 9 * P], f16, name='diagWp')
            for g in range(G):
                for t in range(9):
                    i = g * 9 + t
                    nc.vector.tensor_scalar(
                        diagWp[:, i * P:(i + 1) * P], ident[:],
                        wp[:, g, t:t + 1], None, op0=AL.mult)
            # f_frac * identity (fp16) for the Box tap
            ffrac_bc = perm.tile([P, 1], f32, name='ffrac_bc')
            nc.gpsimd.partition_broadcast(ffrac_bc[:], fscal[:, 0:1])
            factI16 = perm.tile([P, P], f16, name='factI16')
            nc.vector.tensor_scalar(factI16[:], ident[:], ffrac_bc[:, 0:1],
                                    None, op0=AL.mult)

            # ---------------- stat tiles
            xstat = [perm.tile([P, 2 * BL], f32, name=f'xstat{g}')
                     for g in range(G)]
            stat1 = [perm.tile([P, 9], f32, name=f'stat1_{g}')
                     for g in range(G)]
            stat3 = [perm.tile([P, 9], f32, name=f'stat3_{g}')
                     for g in range(G)]
            h1part = [perm.tile([P, 4 * BL], f32, name=f'h1part{g}')
                      for g in range(G)]
            h3part = [perm.tile([P, 4 * BL], f32, name=f'h3part{g}')
                      for g in range(G)]
            Ag = [perm.tile([P, NCORES, 9], f32, name=f'Ag{g}')
                  for g in range(G)]

            def sc1(nm):
                return perm.tile([1, 1], f32, name=nm)

            def bc1(src, nm):
                t = perm.tile([P, 1], f32, name=nm)
                nc.gpsimd.partition_broadcast(t[:], src[:])
                return t

            # scalar math: sum-of-mins/maxes -> quant params (mn, s, 1/s, ...)
            def quant_params(mnsum, mxsum, tag):
                mn = sc1(f'mn_{tag}')
                mx = sc1(f'mx_{tag}')
                nc.vector.tensor_scalar(mn[:], mnsum[:], 1.0 / B_FULL, None,
                                        op0=AL.mult)
                nc.vector.tensor_scalar(mx[:], mxsum[:], 1.0 / B_FULL, None,
                                        op0=AL.mult)
                d = sc1(f'd_{tag}')
                nc.vector.tensor_sub(d[:], mx[:], mn[:])
                s = sc1(f's_{tag}')
                nc.vector.tensor_scalar(s[:], d[:], 1.0 / QMAX, 1e-8,
                                        op0=AL.mult, op1=AL.max)
                inv_s = sc1(f'invs_{tag}')
                nc.vector.reciprocal(inv_s[:], s[:])
                negmn = sc1(f'negmn_{tag}')
                nc.vector.tensor_scalar(negmn[:], mn[:], -1.0, None,
                                        op0=AL.mult)
                bias = sc1(f'bias_{tag}')
                nc.vector.tensor_mul(bias[:], negmn[:], inv_s[:])
                return {'mn': mn, 'mx': mx, 's': s, 'inv_s': inv_s,
                        'negmn': negmn, 'bias': bias}

            # per-sample params from per-(c, core, b) min/max views
            def sample_params(mnviews, mxviews, tag):
                tmn = tpp.tile([B_FULL, G * P], f32, name=f'tmn_{tag}',
                               tag='tp')
                tmx = tpp.tile([B_FULL, G * P], f32, name=f'tmx_{tag}',
                               tag='tp')
                for g in range(G):
                    cmn = perm.tile([P, B_FULL], f32, name=f'cmn_{tag}{g}')
                    cmx = perm.tile([P, B_FULL], f32, name=f'cmx_{tag}{g}')
                    vmn = cmn.rearrange('p (core b) -> p core b', b=BL)
                    vmx = cmx.rearrange('p (core b) -> p core b', b=BL)
                    nc.vector.tensor_scalar(vmn[:], mnviews[g], 1.0, None,
                                            op0=AL.mult)
                    nc.vector.tensor_scalar(vmx[:], mxviews[g], 1.0, None,
                                            op0=AL.mult)
                    nc.tensor.transpose(tmn[:, g * P:(g + 1) * P],
                                        cmn[:], ident[:])
                    nc.tensor.transpose(tmx[:, g * P:(g + 1) * P],
                                        cmx[:], ident[:])
                pm = perm.tile([B_FULL, 2], f32, name=f'pm_{tag}')
                nc.vector.tensor_reduce(pm[:, 0:1], tmn[:], axis=AX.X,
                                        op=AL.min)
                nc.vector.tensor_reduce(pm[:, 1:2], tmx[:], axis=AX.X,
                                        op=AL.max)
                ta = tpp.tile([1, B_FULL], f32, name=f'ta_{tag}', tag='tp')
                tb = tpp.tile([1, B_FULL], f32, name=f'tb_{tag}', tag='tp')
                nc.tensor.transpose(ta[:], pm[:, 0:1],
                                    ident[0:B_FULL, 0:B_FULL])
                nc.tensor.transpose(tb[:], pm[:, 1:2],
                                    ident[0:B_FULL, 0:B_FULL])
                mnsum = sc1(f'mnsum_{tag}')
                mxsum = sc1(f'mxsum_{tag}')
                nc.vector.tensor_reduce(mnsum[:], ta[:], axis=AX.X, op=AL.add)
                nc.vector.tensor_reduce(mxsum[:], tb[:], axis=AX.X, op=AL.add)
                return quant_params(mnsum, mxsum, tag)

            # =================================================================
            # Phase A: load x + per-(c,b) stats
            # =================================================================
            def stat_minmax(src, dst_min, dst_max, scrpool):
                nc.vector.tensor_reduce(dst_min, src[:], axis=AX.X, op=AL.min)
                nc.vector.tensor_reduce(dst_max, src[:], axis=AX.X, op=AL.max)

            xt = {}
            for b in range(BL):
                for g in range(G):
                    t = img.tile([P, IMG], f32, name=f'x{g}_{b}', tag='img')
                    xt[(g, b)] = t
                    nc.sync.dma_start(
                        t[:], x_in[b, g * P:(g + 1) * P].rearrange(
                            'c h w -> c (h w)'))
                    stat_minmax(t, xstat[g][:, b:b + 1],
                                xstat[g][:, BL + b:BL + b + 1], scrp)

            # ---------------- AG1: per-sample min/max (8 floats per core)
            tmin = tpp.tile([BL, G * P], f32, name='tmin1', tag='tp')
            tmax = tpp.tile([BL, G * P], f32, name='tmax1', tag='tp')
            for g in range(G):
                nc.tensor.transpose(tmin[:, g * P:(g + 1) * P],
                                    xstat[g][:, 0:BL], ident[:])
                nc.tensor.transpose(tmax[:, g * P:(g + 1) * P],
                                    xstat[g][:, BL:2 * BL], ident[:])
            ab1 = perm.tile([BL, 2], f32, name='ab1')
            nc.vector.tensor_reduce(ab1[:, 0:1], tmin[:], axis=AX.X, op=AL.min)
            nc.vector.tensor_reduce(ab1[:, 1:2], tmax[:], axis=AX.X, op=AL.max)
            ag1_in = dpool.tile([BL * 2], f32, name='ag1_in')
            ag1_out = dpool.tile([NCORES * BL * 2], f32, name='ag1_out')
            nc.sync.dma_start(ag1_in.rearrange('(b s) -> b s', s=2), ab1[:])
            nc.gpsimd.collective_compute(
                'AllGather', AL.bypass, replica_groups=rg,
                ins=[ag1_in[:].opt()], outs=[ag1_out[:].opt()])
            agb1 = perm.tile([1, NCORES * BL * 2], f32, name='agb1')
            nc.sync.dma_start(agb1[:], ag1_out[None, :])
            v1 = agb1.rearrange('p (cb s) -> p s cb', s=2)
            mnsum_x = sc1('mnsum_x')
            mxsum_x = sc1('mxsum_x')
            nc.vector.tensor_reduce(mnsum_x[:], v1[:, 0, :], axis=AX.X,
                                    op=AL.add)
            nc.vector.tensor_reduce(mxsum_x[:], v1[:, 1, :], axis=AX.X,
                                    op=AL.add)
            qx = quant_params(mnsum_x, mxsum_x, 'x')
            invsx_bc = bc1(qx['inv_s'], 'invsx_bc')
            biasx_bc = bc1(qx['bias'], 'biasx_bc')
            mnx_bc = bc1(qx['mn'], 'mnx_bc')
            negmnx_bc = bc1(qx['negmn'], 'negmnx_bc')
            # S = s_x * s_w  (depthwise evict scale)
            Ssc = sc1('Ssc')
            swn = perm.tile([1, 1], f32, name='swn')
            nc.vector.tensor_scalar(swn[:], fscal[:, 1:2], 1.0, None,
                                    op0=AL.mult)
            nc.vector.tensor_mul(Ssc[:], qx['s'][:], swn[:])
            Sdw_bc = bc1(Ssc, 'Sdw_bc')

            # strip-correction per-channel constants (computed once)
            cstrip = perm.tile([P, 8, G], f32, name='cstrip')
            # 0..3: -mn*w{top,bot,left,right}; 4..7: +mn*w{00,02,20,22}
            for i, k in enumerate((7, 8, 9, 10)):
                nc.vector.tensor_scalar(cstrip[:, i], gp(k),
                                        negmnx_bc[:, 0:1], None, op0=AL.mult)
            for i, k in enumerate((11, 12, 13, 14)):
                nc.vector.tensor_scalar(cstrip[:, 4 + i], gp(k),
                                        mnx_bc[:, 0:1], None, op0=AL.mult)
            # const1 = qdb + mn*wsum ; sum adjust = mn*csum4
            const1 = perm.tile([P, G], f32, name='const1')
            nc.vector.scalar_tensor_tensor(
                const1[:], gp(0), mnx_bc[:, 0:1], gp(1),
                op0=AL.mult, op1=AL.add)
            adj1 = perm.tile([P, G], f32, name='adj1')
            nc.vector.tensor_scalar(adj1[:], gp(15), mnx_bc[:, 0:1], None,
                                    op0=AL.mult)

            # =================================================================
            # Phase B+C: quantize x -> k (u8->f16), Box, depthwise PE, evict
            # =================================================================
            h1 = {}
            with (
                tc.tile_pool(name='bpool', bufs=3) as bpool,
                tc.tile_pool(name='kxp', bufs=2) as kxp,
                tc.tile_pool(name='cva', bufs=3, space='PSUM') as cvap,
                tc.tile_pool(name='cvb', bufs=1, space='PSUM') as cvbp,
            ):
                deferred = []
                for b in range(BL):
                    for g in range(G):
                        ku = kxp.tile([P, IMG], u8, name=f'ku{g}_{b}',
                                      tag='ku')
                        nc.scalar.activation(ku[:], xt[(g, b)][:], AF.Relu,
                                             bias=biasx_bc[:, 0:1],
                                             scale=invsx_bc[:, 0:1])
                        kh = bpool.tile([P, PADW, PADW], f16,
                                        name=f'kh{g}_{b}', tag='kh')
                        # zero borders (rows 0,57; cols 0,57)
                        nc.gpsimd.memset(kh[:, 0, :], 0.0)
                        nc.gpsimd.memset(kh[:, 57, :], 0.0)
                        nc.gpsimd.memset(kh[:, 1:57, 0], 0.0)
                        nc.gpsimd.memset(kh[:, 1:57, 57], 0.0)
                        nc.scalar.activation(
                            kh[:, 1:57, 1:57],
                            ku.rearrange('p (h w) -> p h w', h=HH),
                            AF.Identity, bias=0.0, scale=1.0)
                        # V = vertical 3-sum of kh; the horizontal 3-sum of
                        # the f_frac*Box term rides the PE as 3 extra taps
                        V = bpool.tile([P, HH, PADW], f16, name=f'V{g}_{b}',
                                       tag='bx')
                        nc.vector.tensor_tensor(V[:], kh[:, 0:56, :],
                                                kh[:, 1:57, :], op=AL.add)
                        nc.vector.tensor_tensor(V[:], V[:], kh[:, 2:58, :],
                                                op=AL.add)
                        box = bpool.tile([P, HH, HH], f16, name=f'bx{g}_{b}',
                                         tag='bx')
                        nc.vector.tensor_tensor(box[:], V[:, :, 0:56],
                                                V[:, :, 1:57], op=AL.add)
                        nc.vector.tensor_tensor(box[:], box[:],
                                                V[:, :, 2:58], op=AL.add)
                        boxf = box.rearrange('p h w -> p (h w)')
                        # depthwise: 7 row-blocks x (9 diag + box tap);
                        # PSUM blocks packed 4+3 into two multi-bank tiles
                        h1t = img.tile([P, IMG], f32, name=f'h1_{g}_{b}',
                                       tag='img')
                        h1[(g, b)] = h1t
                        pst = [cvap.tile([P, 2, 512], f32,
                                         name=f'cva{g}{b}{i}', tag='cva')
                               for i in range(3)]
                        psd = cvbp.tile([P, 512], f32, name=f'cvb{g}{b}',
                                        tag='cvb')
                        for rb in range(NBLK):
                            ps = (pst[rb // 2][:, rb % 2, 0:BLKW] if rb < 6
                                  else psd[:, 0:BLKW])
                            for t in range(9):
                                di, dj = t // 3, t % 3
                                lhs = diagWp[:, (g * 9 + t) * P:
                                             (g * 9 + t + 1) * P]
                                rhs = kh[:, 8 * rb + di:8 * rb + di + 8,
                                         dj:dj + 56]
                                nc.tensor.matmul(ps, lhs, rhs,
                                                 start=(t == 0), stop=False)
                            nc.tensor.matmul(
                                ps, factI16[:],
                                boxf[:, rb * BLKW:(rb + 1) * BLKW],
                                start=False, stop=True)
                        h1v3 = h1t.rearrange('p (r w) -> p r w', w=BLKW)
                        for i in range(3):
                            nc.scalar.activation(
                                h1v3[:, 2 * i:2 * i + 2, :],
                                pst[i][:, :, 0:BLKW],
                                AF.Identity, bias=const1[:, g:g + 1],
                                scale=Sdw_bc[:, 0:1],
                                accum_out=h1part[g][:, 4 * b + i:
                                                    4 * b + i + 1])
                        nc.scalar.activation(
                            h1v3[:, 6:7, :], psd[:, 0:BLKW],
                            AF.Identity, bias=const1[:, g:g + 1],
                            scale=Sdw_bc[:, 0:1],
                            accum_out=h1part[g][:, 4 * b + 3:4 * b + 4])

                        def post_c(g=g, b=b, h1t=h1t):
                            # border strip corrections
                            h1v = h1t.rearrange('p (h w) -> p h w', h=HH)
                            nc.vector.tensor_scalar(
                                h1t[:, 0:56], h1t[:, 0:56],
                                cstrip[:, 0, g:g + 1], None, op0=AL.add)
                            nc.vector.tensor_scalar(
                                h1t[:, 3080:3136], h1t[:, 3080:3136],
                                cstrip[:, 1, g:g + 1], None, op0=AL.add)
                            nc.vector.tensor_scalar(
                                h1v[:, :, 0], h1v[:, :, 0],
                                cstrip[:, 2, g:g + 1], None, op0=AL.add)
                            nc.vector.tensor_scalar(
                                h1v[:, :, 55], h1v[:, :, 55],
                                cstrip[:, 3, g:g + 1], None, op0=AL.add)
                            for ci, off in enumerate((0, 55, 3080, 3135)):
                                nc.vector.tensor_scalar(
                                    h1t[:, off:off + 1],
                                    h1t[:, off:off + 1],
                                    cstrip[:, 4 + ci, g:g + 1], None,
                                    op0=AL.add)
                            stat_minmax(h1t, stat1[g][:, b:b + 1],
                                        stat1[g][:, BL + b:BL + b + 1],
                                        scrp)
                        deferred.append(post_c)
                        if len(deferred) > 1:
                            deferred.pop(0)()
                while deferred:
                    deferred.pop(0)()

                # channel sums (+ analytic border-correction adjustment)
                for g in range(G):
                    nc.vector.tensor_reduce(stat1[g][:, 8:9], h1part[g][:],
                                            axis=AX.X, op=AL.add)
                    nc.vector.tensor_tensor(stat1[g][:, 8:9],
                                            stat1[g][:, 8:9],
                                            adj1[:, g:g + 1], op=AL.add)

            # =================================================================
            # AG2: per-(c,b) h1 min/max + per-c sums
            # =================================================================
            ag2_in = dpool.tile([G * P * 9], f32, name='ag2_in')
            ag2_out = dpool.tile([NCORES * G * P * 9], f32, name='ag2_out')
            v2i = ag2_in.rearrange('(g c f) -> g c f', g=G, c=P)
            for g in range(G):
                nc.sync.dma_start(v2i[g], stat1[g][:])
            nc.gpsimd.collective_compute(
                'AllGather', AL.bypass, replica_groups=rg,
                ins=[ag2_in[:].opt()], outs=[ag2_out[:].opt()])
            v2o = ag2_out.rearrange('(core g c f) -> g c core f',
                                    core=NCORES, g=G, c=P)
            for g in range(G):
                nc.sync.dma_start(Ag[g][:], v2o[g])

            # ---- q1 params (per-sample over all 256 channels)
            q1 = sample_params([Ag[g][:, :, 0:BL] for g in range(G)],
                               [Ag[g][:, :, BL:2 * BL] for g in range(G)],
                               'h1')
            invs1_bc = bc1(q1['inv_s'], 'invs1_bc')
            bias1_bc = bc1(q1['bias'], 'bias1_bc')
            s1_bc = bc1(q1['s'], 's1_bc')
            mn1_bc = bc1(q1['mn'], 'mn1_bc')

            # ---- RangeBN scale from chunk (batch-pair) stats
            def rangebn_scale(AgT, invs_bc, bias_bc, s_bc, mn_bc, tag):
                scpk = perm.tile([P, G], f32, name=f'scpk_{tag}')
                for g in range(G):
                    cmin = perm.tile([P, NCHUNKS], f32, name=f'cmin_{tag}{g}')
                    cmax = perm.tile([P, NCHUNKS], f32, name=f'cmax_{tag}{g}')
                    cminv = cmin.rearrange('p (core pr) -> p core pr', pr=2)
                    cmaxv = cmax.rearrange('p (core pr) -> p core pr', pr=2)
                    nc.vector.tensor_tensor(cminv[:], AgT[g][:, :, 0:BL:2],
                                            AgT[g][:, :, 1:BL:2], op=AL.min)
                    nc.vector.tensor_tensor(cmaxv[:],
                                            AgT[g][:, :, BL:2 * BL:2],
                                            AgT[g][:, :, BL + 1:2 * BL:2],
                                            op=AL.max)
                    # quantize chunk stats (monotone): k then value form
                    kq = perm.tile([P, 2 * NCHUNKS], u8, name=f'kq_{tag}{g}')
                    nc.scalar.activation(kq[:, 0:NCHUNKS], cmin[:], AF.Relu,
                                         bias=bias_bc[:, 0:1],
                                         scale=invs_bc[:, 0:1])
                    nc.scalar.activation(kq[:, NCHUNKS:], cmax[:], AF.Relu,
                                         bias=bias_bc[:, 0:1],
                                         scale=invs_bc[:, 0:1])
                    vq = perm.tile([P, 2 * NCHUNKS], f32, name=f'vq_{tag}{g}')
                    nc.vector.tensor_scalar(vq[:], kq[:], s_bc[:, 0:1],
                                            mn_bc[:, 0:1], op0=AL.mult,
                                            op1=AL.add)
                    mm = perm.tile([P, 2], f32, name=f'mm_{tag}{g}')
                    nc.vector.tensor_reduce(mm[:, 0:1], vq[:, 0:NCHUNKS],
                                            axis=AX.X, op=AL.add)
                    nc.vector.tensor_reduce(mm[:, 1:2], vq[:, NCHUNKS:],
                                            axis=AX.X, op=AL.add)
                    d = perm.tile([P, 1], f32, name=f'dmm_{tag}{g}')
                    nc.vector.tensor_sub(d[:], mm[:, 1:2], mm[:, 0:1])
                    # d = (mean_max-mean_min)*scale_fix + eps  (note /16)
                    nc.vector.tensor_scalar(d[:], d[:],
                                            SCALE_FIX / NCHUNKS, EPS,
                                            op0=AL.mult, op1=AL.add)
                    nc.vector.reciprocal(scpk[:, g:g + 1], d[:])
                # per-tensor quantize of the 256 scales
                tq = tpp.tile([1, G * P], f32, name=f'tq_{tag}', tag='tp')
                for g in range(G):
                    nc.tensor.transpose(tq[:, g * P:(g + 1) * P],
                                        scpk[:, g:g + 1], ident[:])
                smn = sc1(f'smn_{tag}')
                smx = sc1(f'smx_{tag}')
                nc.vector.tensor_reduce(smn[:], tq[:], axis=AX.X, op=AL.min)
                nc.vector.tensor_reduce(smx[:], tq[:], axis=AX.X, op=AL.max)
                dd = sc1(f'sd_{tag}')
                nc.vector.tensor_sub(dd[:], smx[:], smn[:])
                ss = sc1(f'ss_{tag}')
                nc.vector.tensor_scalar(ss[:], dd[:], 1.0 / QMAX, 1e-8,
                                        op0=AL.mult, op1=AL.max)
                invss = sc1(f'invss_{tag}')
                nc.vector.reciprocal(invss[:], ss[:])
                negsmn = sc1(f'negsmn_{tag}')
                nc.vector.tensor_scalar(negsmn[:], smn[:], -1.0, None,
                                        op0=AL.mult)
                bss = sc1(f'bss_{tag}')
                nc.vector.tensor_mul(bss[:], negsmn[:], invss[:])
                invss_bc = bc1(invss, f'invss_bc_{tag}')
                bss_bc = bc1(bss, f'bss_bc_{tag}')
                ss_bc = bc1(ss, f'ss_bc_{tag}')
                smn_bc = bc1(smn, f'smn_bc_{tag}')
                kqs = perm.tile([P, G], u8, name=f'kqs_{tag}')
                nc.scalar.activation(kqs[:], scpk[:], AF.Relu,
                                     bias=bss_bc[:, 0:1],
                                     scale=invss_bc[:, 0:1])
                nc.vector.tensor_scalar(scpk[:], kqs[:], ss_bc[:, 0:1],
                                        smn_bc[:, 0:1], op0=AL.mult,
                                        op1=AL.add)
                return scpk

            qscale1 = rangebn_scale(Ag, invs1_bc, bias1_bc, s1_bc, mn1_bc,
                                    'bn1')
            A1 = perm.tile([P, G], f32, name='A1')
            nc.vector.tensor_mul(A1[:], qscale1[:], gp(2))
            cA1 = perm.tile([P, G], f32, name='cA1')
            nc.vector.tensor_scalar(cA1[:], A1[:], s1_bc[:, 0:1], None,
                                    op0=AL.mult)
            # mean1 = (sum over cores of per-core h1 sums) / N_TOT
            mean1 = perm.tile([P, G], f32, name='mean1')
            for g in range(G):
                nc.vector.tensor_reduce(mean1[:, g:g + 1], Ag[g][:, :, 8],
                                        axis=AX.X, op=AL.add)
            nc.vector.tensor_scalar(mean1[:], mean1[:], 1.0 / N_TOT, None,
                                    op0=AL.mult)
            cB1 = perm.tile([P, G], f32, name='cB1')
            nc.vector.tensor_scalar(cB1[:], mean1[:], -1.0, mn1_bc[:, 0:1],
                                    op0=AL.mult, op1=AL.add)
            nc.vector.tensor_mul(cB1[:], cB1[:], A1[:])
            nc.vector.tensor_add(cB1[:], cB1[:], gp(3))

            # ---- analytic qm(h2) bounds: transform Ag min/max in place
            for g in range(G):
                kb = perm.tile([P, 2 * BL * NCORES], u8, name=f'kb_{g}')
                kbv = kb.rearrange('p (core f) -> p core f', f=2 * BL)
                nc.scalar.activation(kbv[:], Ag[g][:, :, 0:2 * BL], AF.Relu,
                                     bias=bias1_bc[:, 0:1],
                                     scale=invs1_bc[:, 0:1])
                nc.scalar.activation(Ag[g][:, :, 0:2 * BL], kbv[:], AF.Relu,
                                     bias=cB1[:, g:g + 1],
                                     scale=cA1[:, g:g + 1])
            q2 = sample_params([Ag[g][:, :, 0:BL] for g in range(G)],
                               [Ag[g][:, :, BL:2 * BL] for g in range(G)],
                               'h2')
            invs2_bc = bc1(q2['inv_s'], 'invs2_bc')
            mn2_bc = bc1(q2['mn'], 'mn2_bc')
            s2_bc = bc1(q2['s'], 's2_bc')

            # fused D/E coefficients and pointwise weight prep
            a2t = perm.tile([P, G], f32, name='a2t')
            nc.vector.tensor_scalar(a2t[:], cA1[:], invs2_bc[:, 0:1], None,
                                    op0=AL.mult)
            b2t = perm.tile([P, G], f32, name='b2t')
            nc.vector.tensor_scalar(b2t[:], cB1[:], mn2_bc[:, 0:1],
                                    invs2_bc[:, 0:1], op0=AL.subtract,
                                    op1=AL.mult)
            pwTs = perm.tile([P, G, 256], f16, name='pwTs')
            nc.scalar.activation(pwTs[:], pwT[:], AF.Identity, bias=0.0,
                                 scale=s2_bc[:, 0:1])
            const3 = perm.tile([P, G], f32, name='const3')
            nc.vector.tensor_scalar(const3[:], gp(6), mn2_bc[:, 0:1], None,
                                    op0=AL.mult)

            # =================================================================
            # Phase D/E/F: h1 -> k1 -> k2 -> pointwise -> h3 (in SBUF)
            # =================================================================
            h3 = {}
            with (
                tc.tile_pool(name='k1p', bufs=3) as k1p,
                tc.tile_pool(name='k2up', bufs=2) as k2up,
                tc.tile_pool(name='k2fp', bufs=4) as k2fp,
                tc.tile_pool(name='pwa', bufs=3, space='PSUM') as pwap,
                tc.tile_pool(name='pwb', bufs=1, space='PSUM') as pwbp,
            ):
                deferred3 = []
                for b in range(BL):
                    k2f = {}
                    for g in range(G):
                        k1t = k1p.tile([P, IMG], u8, name=f'k1_{g}_{b}',
                                       tag='k1')
                        if g == 0:
                            nc.scalar.activation(k1t[:], h1[(g, b)][:],
                                                 AF.Relu,
                                                 bias=bias1_bc[:, 0:1],
                                                 scale=invs1_bc[:, 0:1])
                        else:
                            nc.vector.tensor_scalar(k1t[:], h1[(g, b)][:],
                                                    invs1_bc[:, 0:1],
                                                    bias1_bc[:, 0:1],
                                                    op0=AL.mult, op1=AL.add)
                        k2u = k2up.tile([P, IMG], u8, name=f'k2u_{g}_{b}',
                                        tag='k2u')
                        nc.scalar.activation(k2u[:], k1t[:], AF.Relu,
                                             bias=b2t[:, g:g + 1],
                                             scale=a2t[:, g:g + 1])
                        k2ft = k2fp.tile([P, IMG], f16, name=f'k2f_{g}_{b}',
                                         tag='k2f')
                        k2f[g] = k2ft
                        nc.vector.tensor_scalar(k2ft[:], k2u[:], 1.0, None,
                                                op0=AL.mult)
                    for cg in range(G):
                        h3t = img.tile([P, IMG], f32, name=f'h3_{cg}_{b}',
                                       tag='img')
                        h3[(cg, b)] = h3t
                        pst = [pwap.tile([P, 2, 512], f32,
                                         name=f'pwa{cg}{b}{i}', tag='pwa')
                               for i in range(3)]
                        psd = pwbp.tile([P, 512], f32, name=f'pwb{cg}{b}',
                                        tag='pwb')
                        for rb in range(NBLK):
                            ps = (pst[rb // 2][:, rb % 2, 0:BLKW] if rb < 6
                                  else psd[:, 0:BLKW])
                            for kg in range(G):
                                lhs = pwTs[:, kg, cg * P:(cg + 1) * P]
                                nc.tensor.matmul(
                                    ps, lhs,
                                    k2f[kg][:, rb * BLKW:(rb + 1) * BLKW],
                                    start=(kg == 0), stop=(kg == 1))
                        h3v3 = h3t.rearrange('p (r w) -> p r w', w=BLKW)
                        for i in range(3):
                            nc.scalar.activation(
                                h3v3[:, 2 * i:2 * i + 2, :],
                                pst[i][:, :, 0:BLKW],
                                AF.Identity, bias=const3[:, cg:cg + 1],
                                scale=1.0,
                                accum_out=h3part[cg][:, 4 * b + i:
                                                     4 * b + i + 1])
                        nc.scalar.activation(
                            h3v3[:, 6:7, :], psd[:, 0:BLKW],
                            AF.Identity, bias=const3[:, cg:cg + 1],
                            scale=1.0,
                            accum_out=h3part[cg][:, 4 * b + 3:4 * b + 4])

                        def post_f(cg=cg, b=b, h3t=h3t):
                            stat_minmax(h3t, stat3[cg][:, b:b + 1],
                                        stat3[cg][:, BL + b:BL + b + 1],
                                        scrp)
                        deferred3.append(post_f)
                        if len(deferred3) > 1:
                            deferred3.pop(0)()
                while deferred3:
                    deferred3.pop(0)()
                for g in range(G):
                    nc.vector.tensor_reduce(stat3[g][:, 8:9], h3part[g][:],
                                            axis=AX.X, op=AL.add)

            # =================================================================
            # AG5 + BN2 math
            # =================================================================
            ag5_in = dpool.tile([G * P * 9], f32, name='ag5_in')
            ag5_out = dpool.tile([NCORES * G * P * 9], f32, name='ag5_out')
            v5i = ag5_in.rearrange('(g c f) -> g c f', g=G, c=P)
            for g in range(G):
                nc.sync.dma_start(v5i[g], stat3[g][:])
            nc.gpsimd.collective_compute(
                'AllGather', AL.bypass, replica_groups=rg,
                ins=[ag5_in[:].opt()], outs=[ag5_out[:].opt()])
            v5o = ag5_out.rearrange('(core g c f) -> g c core f',
                                    core=NCORES, g=G, c=P)
            for g in range(G):
                nc.sync.dma_start(Ag[g][:], v5o[g])

            q3 = sample_params([Ag[g][:, :, 0:BL] for g in range(G)],
                               [Ag[g][:, :, BL:2 * BL] for g in range(G)],
                               'h3')
            invs3_bc = bc1(q3['inv_s'], 'invs3_bc')
            bias3_bc = bc1(q3['bias'], 'bias3_bc')
            s3_bc = bc1(q3['s'], 's3_bc')
            mn3_bc = bc1(q3['mn'], 'mn3_bc')
            qscale3 = rangebn_scale(Ag, invs3_bc, bias3_bc, s3_bc, mn3_bc,
                                    'bn2')
            A3 = perm.tile([P, G], f32, name='A3')
            nc.vector.tensor_mul(A3[:], qscale3[:], gp(4))
            cA3 = perm.tile([P, G], f32, name='cA3')
            nc.vector.tensor_scalar(cA3[:], A3[:], s3_bc[:, 0:1], None,
                                    op0=AL.mult)
            mean3 = perm.tile([P, G], f32, name='mean3')
            for g in range(G):
                nc.vector.tensor_reduce(mean3[:, g:g + 1], Ag[g][:, :, 8],
                                        axis=AX.X, op=AL.add)
            nc.vector.tensor_scalar(mean3[:], mean3[:], 1.0 / N_TOT, None,
                                    op0=AL.mult)
            cB3 = perm.tile([P, G], f32, name='cB3')
            nc.vector.tensor_scalar(cB3[:], mean3[:], -1.0, mn3_bc[:, 0:1],
                                    op0=AL.mult, op1=AL.add)
            nc.vector.tensor_mul(cB3[:], cB3[:], A3[:])
            nc.vector.tensor_add(cB3[:], cB3[:], gp(5))

            # =================================================================
            # Phase G/H: h3 -> k3 -> out
            # =================================================================
            with tc.tile_pool(name='k3p', bufs=3) as k3p:
                for b in range(BL):
                    for cg in range(G):
                        k3t = k3p.tile([P, IMG], u8, name=f'k3_{cg}_{b}',
                                       tag='k3')
                        nc.vector.tensor_scalar(k3t[:], h3[(cg, b)][:],
                                                invs3_bc[:, 0:1],
                                                bias3_bc[:, 0:1],
                                                op0=AL.mult, op1=AL.add)
                        ot = img.tile([P, IMG], f32, name=f'out_{cg}_{b}',
                                      tag='img')
                        nc.scalar.activation(ot[:], k3t[:], AF.Relu,
                                             bias=cB3[:, cg:cg + 1],
                                             scale=cA3[:, cg:cg + 1])
                        nc.sync.dma_start(
                            out_d[b, cg * P:(cg + 1) * P].rearrange(
                                'c h w -> c (h w)'), ot[:])

    nc.compile()
    return nc


def _host_consts(dw_w, dw_b, bn1_w, bn1_b, pw_w, bn2_w, bn2_b):
    wcodes, s_w, mn_w = _host_quant_codes(np.asarray(dw_w).reshape(256, 9))
    qdw = (wcodes * s_w + mn_w).astype(np.float32)
    f = float(mn_w) / float(s_w)
    f_int = float(np.round(f))
    f_frac = np.float32(f - f_int)
    wp = (wcodes + np.float32(f_int)).astype(np.float32)

    qdb = _host_quant(dw_b)
    qpw = _host_quant(np.asarray(pw_w).reshape(256, 256))
    qbn1w = _host_quant(bn1_w)
    qbn2w = _host_quant(bn2_w)
    wsum = qdw.sum(axis=1, dtype=np.float32)
    wtop = qdw[:, 0:3].sum(axis=1, dtype=np.float32)
    wbot = qdw[:, 6:9].sum(axis=1, dtype=np.float32)
    wleft = qdw[:, (0, 3, 6)].sum(axis=1, dtype=np.float32)
    wright = qdw[:, (2, 5, 8)].sum(axis=1, dtype=np.float32)
    w00, w02, w20, w22 = qdw[:, 0], qdw[:, 2], qdw[:, 6], qdw[:, 8]
    csum4 = (4.0 * (-56.0 * (wtop + wbot + wleft + wright)
                    + (w00 + w02 + w20 + w22))).astype(np.float32)
    pwsum = qpw.sum(axis=1, dtype=np.float32)
    pwT = np.ascontiguousarray(qpw.T.reshape(G, P, 256)).astype(np.float32)

    def gpr(a):
        return np.asarray(a, np.float32).reshape(G, P)

    gpk = np.stack([gpr(wsum), gpr(qdb), gpr(qbn1w), gpr(bn1_b),
                    gpr(qbn2w), gpr(bn2_b), gpr(pwsum), gpr(wtop),
                    gpr(wbot), gpr(wleft), gpr(wright), gpr(w00),
                    gpr(w02), gpr(w20), gpr(w22), gpr(csum4)], axis=0)
    return {
        'ident': np.eye(P, dtype=np.float32),
        'wp': np.ascontiguousarray(wp.reshape(G, P, 9)),
        'gp': np.ascontiguousarray(gpk),
        'pwT': pwT,
        'fscal': np.array([[f_frac, s_w]], dtype=np.float32),
    }


def make_in_maps(x, dw_w, dw_b, bn1_w, bn1_b, pw_w, bn2_w, bn2_b):
    x = np.asarray(x, np.float32)
    consts = _host_consts(dw_w, dw_b, bn1_w, bn1_b, pw_w, bn2_w, bn2_b)
    in_maps = []
    for c in range(NCORES):
        m = dict(consts)
        m['x'] = np.ascontiguousarray(x[c * BL:(c + 1) * BL])
        in_maps.append(m)
    return in_maps


def get_program(limit=7):
    if limit not in _PROGRAM_CACHE:
        _PROGRAM_CACHE[limit] = build_program(limit)
    return _PROGRAM_CACHE[limit]


def kernel(**inputs):
    from concourse.bass_utils import run_bass_kernel_spmd
    nc = get_program()
    in_maps = make_in_maps(**inputs)
    res = run_bass_kernel_spmd(nc, in_maps, core_ids=list(range(NCORES)))
    out = np.concatenate([res.results[i]['out'] for i in range(NCORES)],
                         axis=0)
    return out.astype(np.float32)
